# revision 1
# baseline (speedup 1.0000x reference)
"""Trainium2 Bass kernel for nn_Block_30107720745811 (dense transformer block).

B=4, S=1024, H=1024, NH=16. 8 NeuronCores, zero-communication sharding:
core c computes batch b=c//2, query rows (c%2)*512:(c%2)*512+512.
K/V projections are duplicated within each batch pair (no collectives).

All activations live transposed [feature, token] in SBUF; weights stream
from HBM in natural [in, out] layout as matmul stationary operands.
Matmuls run in float32r (full PE rate for N>=256, ~tf32 precision).
The softmax denominator rides along the exp@V matmul as a ones column of V.
"""
import numpy as np
import concourse.bass as bass
import concourse.tile as tile
import bass_rust
from concourse import mybir
from concourse import bass_utils
from concourse.alu_op_type import AluOpType as OP

AF = mybir.ActivationFunctionType
F32 = mybir.dt.float32
F32R = mybir.dt.float32r

B, S, H, NH = 4, 1024, 1024, 16
D = H // NH          # 64
P = 128
T = 512              # query tokens per core
KC = H // P          # 8 feature chunks
FC = 4 * H // P      # 32 ffn hidden chunks
HPC = P // D         # heads per feature chunk = 2
INF = 1e10
EPS = 1e-5
SCALE = 8.0          # sqrt(D)

# vec tensor column map
C_SBQ, C_SBK, C_SBO = 0, 8, 16
C_CBQ, C_CBK, C_CBO = 24, 32, 40
C_SAB, C_CAB = 48, 56
C_G, C_B = 64, 72
C_B1, C_B2, C_EPS = 80, 112, 120
C_NW1 = 121
NVEC = 153

MAX_WAITS = 1


def _legalize_waits(nc, max_waits=MAX_WAITS):
    """Split >max_waits semaphore waits into preceding same-engine NOPs
    (this walrus build allows only one sync wait per instruction)."""
    n_split = 0
    for f in nc.m.functions:
        for blk in f.blocks:
            out = []
            for ins in blk.instructions:
                si = getattr(ins, "sync_info", None)
                if si is not None and si.on_wait and len(si.on_wait) > max_waits:
                    waits = list(si.on_wait)
                    extra, keep = waits[:-max_waits], waits[-max_waits:]
                    for j in range(0, len(extra), max_waits):
                        out.append(mybir.InstNoOp(
                            name=f"{ins.name}-lw{j}",
                            engine=ins.engine,
                            sync_info=mybir.SyncInfo(
                                on_wait=extra[j:j + max_waits], on_update=[]),
                            bass_nofuse=True,
                        ))
                    ins.sync_info = mybir.SyncInfo(
                        on_wait=keep, on_update=list(si.on_update))
                    n_split += 1
                out.append(ins)
            blk.instructions = out
    return n_split


def _build(dbg=False):
    nc = bass.Bass("TRN2", target_bir_lowering=False, debug=False,
                   dynamic_dma_scratch_size=8192)

    def din(name, shape, dt=F32R):
        return nc.dram_tensor(name, shape, dt, kind="ExternalInput").ap()

    xq_d = din("xqT", [H, T])            # query-side hidden, transposed
    xk_d = din("xkT", [H, S])            # full hidden (self K/V), transposed
    xc_d = din("xcT", [H, S])            # full cross hidden, transposed
    w_names = ["sWq", "sWk", "sWv", "sWo", "cWq", "cWk", "cWv", "cWo"]
    w_d = {n: din(n, [H, H]) for n in w_names}
    w1_d = din("W1", [H, 4 * H])
    w2_d = din("W2", [4 * H, H])
    vec_d = din("vec", [P, NVEC], F32)
    ones2_d = din("ones2", [P, P])
    out_d = nc.dram_tensor("out", [H, T], F32, kind="ExternalOutput").ap()
    dbg_d = {}
    if dbg:
        for n, shape in [("d_qT", [H, T]), ("d_kT", [H, S]), ("d_v", [S, H]),
                         ("d_at", [H, T]), ("d_sa", [H, T]), ("d_snn", [H, T]),
                         ("d_h", [H, T]), ("d_u", [4 * H, T])]:
            dbg_d[n] = nc.dram_tensor(n, shape, F32, kind="ExternalOutput").ap()

    with (
        tile.TileContext(nc) as tc,
        nc.allow_low_precision(reason="fp32r activations feed matmuls"),
        tc.tile_pool(name="glob", bufs=1) as glob,
        tc.tile_pool(name="ps", bufs=1, space="PSUM") as ps,
        tc.tile_pool(name="drs", bufs=1, space="DRAM") as drs,
    ):
        # ---- constants / vectors ----
        vec = glob.tile([P, NVEC], F32, tag="vec")
        nc.sync.dma_start(vec[:], vec_d[:])
        ones2 = glob.tile([P, P], F32R, tag="ones2")
        nc.sync.dma_start(ones2[:], ones2_d[:])
        xq = glob.tile([P, KC, T], F32R, tag="xq")

        def load_xfull(pool, src_d):
            """Load a [H, S] transposed activation in 4 chunked DMAs."""
            t = pool.tile([P, KC, S], F32R, tag="xfull")
            r = src_d.rearrange("(c p) t -> p c t", p=P)
            for j in range(4):
                nc.sync.dma_start(t[:, 2 * j:2 * j + 2, :],
                                  r[:, 2 * j:2 * j + 2, :])
            return t

        def ln_sums_start():
            psS = ps.tile([1, T], F32, tag="d", bufs=2)
            psQ = ps.tile([1, T], F32, tag="d", bufs=2)
            return psS, psQ

        def ln_sums_chunk(pool, acc, src_chunk, m):
            psS, psQ = acc
            nc.tensor.matmul(psS[:], ones2[:, 0:1], src_chunk,
                             start=(m == 0), stop=(m == KC - 1),
                             skip_group_check=True)
            sq = pool.tile([P, T], F32R, tag="sq", bufs=2)
            nc.scalar.activation(sq[:], src_chunk, AF.Square)
            nc.tensor.matmul(psQ[:], ones2[:, 0:1], sq[:],
                             start=(m == 0), stop=(m == KC - 1),
                             skip_group_check=True)

        def ln_finish(pool, acc, src, gcol, bcol, dbg_name=None, out_dma=None,
                      fused_copies=None):
            psS, psQ = acc
            mean = pool.tile([1, T], F32, tag="lnv", bufs=3)
            nc.scalar.mul(mean[:], psS[:], 1.0 / H)
            ex2 = pool.tile([1, T], F32, tag="lnv", bufs=3)
            nc.scalar.mul(ex2[:], psQ[:], 1.0 / H)
            var = pool.tile([1, T], F32, tag="lnv", bufs=3)
            nc.vector.tensor_tensor(var[:], mean[:], mean[:], op=OP.mult)
            nc.vector.tensor_tensor(var[:], ex2[:], var[:], op=OP.subtract)
            lv = pool.tile([1, T], F32, tag="lnv", bufs=3)
            nc.scalar.activation(lv[:], var[:], AF.Ln,
                                 bias=vec[0:1, C_EPS:C_EPS + 1])
            rstd = pool.tile([1, T], F32R, tag="lnr", bufs=2)
            nc.scalar.activation(rstd[:], lv[:], AF.Exp, scale=-0.5)
            meanr = pool.tile([1, T], F32R, tag="lnr", bufs=2)
            nc.vector.tensor_copy(meanr[:], mean[:])
            psA = ps.tile([P, T], F32, tag="ss", bufs=2)
            nc.tensor.matmul(psA[:], ones2[0:1, :], rstd[:], start=True,
                             stop=True)
            psC = ps.tile([P, T], F32, tag="ss", bufs=2)
            nc.tensor.matmul(psC[:], ones2[0:1, :], meanr[:], start=True,
                             stop=True)
            bcast_sb = None
            if fused_copies is not None:
                mb, ab = fused_copies
                nc.scalar.copy(mb[:], psC[:])
                nc.scalar.copy(ab[:], psA[:])
                bcast_sb = (mb, ab)
            dst = glob.tile([P, KC, T], F32R, tag="lnq")
            for m in range(KC):
                t1 = pool.tile([P, T], F32, tag="rb", bufs=2)
                nc.vector.scalar_tensor_tensor(t1[:], src.bitcast(F32)[:, m, :],
                                               0.0, psC[:], op0=OP.bypass,
                                               op1=OP.subtract)
                nc.vector.scalar_tensor_tensor(t1[:], t1[:], 0.0, psA[:],
                                               op0=OP.bypass, op1=OP.mult)
                nc.scalar.activation(dst[:, m, :], t1[:], AF.Identity,
                                     bias=vec[:, bcol + m:bcol + m + 1],
                                     scale=vec[:, gcol + m:gcol + m + 1])
                if out_dma is not None:
                    nc.sync.dma_start(out_dma[m * P:(m + 1) * P, :],
                                      dst.bitcast(F32)[:, m, :])
            if dbg and dbg_name:
                nc.sync.dma_start(
                    dbg_d[dbg_name].rearrange("(c p) t -> p c t", p=P),
                    dst.bitcast(F32)[:])
            return dst

        def attention(pool, q_src, x_kv, Wq, Wk, Wv, Wo, qb_col, kb_col,
                      ob_col, ab_col, dbg_prefix=None, post_v_hook=None,
                      ln_acc=None):
            """Full MHA incl. out-proj + residual(xq): returns sa [P, KC, T]
            f32r (glob tag 'res')."""
            # V projection, natural [token, head, dim+ones] layout
            vt = pool.tile([P, KC, NH, D + 1], F32R, tag="vt")
            for i in range(KC):
                nc.gpsimd.dma_start(vt[:, i, :, D:D + 1], ones2[:, 0:NH])
            NS = H // 4  # 256
            NHS = NS // D  # heads per slice = 4
            for n in range(4):
                wv = pool.tile([P, KC, NS], F32R, tag="wmov", bufs=2)
                nc.sync.dma_start(
                    wv[:], Wv.rearrange("(c p) n -> p c n", p=P)
                    [:, :, n * NS:(n + 1) * NS])
                if n == 0 and post_v_hook is not None:
                    post_v_hook()
                for i in range(KC):
                    pv = ps.tile([P, NS], F32, tag="mm", bufs=2)
                    for k in range(KC):
                        nc.tensor.matmul(pv[:],
                                         x_kv[:, k, i * P:(i + 1) * P],
                                         wv[:, k, :],
                                         start=(k == 0), stop=(k == KC - 1))
                    nc.vector.tensor_copy(
                        vt[:, i, n * NHS:(n + 1) * NHS, 0:D],
                        pv.rearrange("p (h d) -> p h d", d=D)[:])
            if dbg and dbg_prefix == "s":
                for i in range(KC):
                    nc.sync.dma_start(
                        dbg_d["d_v"][i * P:(i + 1) * P, :]
                        .rearrange("p (h d) -> p h d", d=D),
                        vt.bitcast(F32)[:, i, :, 0:D])

            at = pool.tile([P, KC, T], F32R, tag="at")
            wo_tiles = {}
            for mp in range(0, KC, 2):
                if mp == KC - 2:
                    wo0 = pool.tile([P, KC, 2 * P], F32R, tag="wst", bufs=3)
                    nc.sync.dma_start(
                        wo0[:], Wo.rearrange("(c p) m -> p c m", p=P)
                        [:, :, 0:2 * P])
                    wo_tiles[0] = wo0
                # paired weight loads (2 m-chunks per DMA)
                wq = pool.tile([P, KC, 2 * P], F32R, tag="wst", bufs=3)
                nc.sync.dma_start(
                    wq[:], Wq.rearrange("(c p) m -> p c m", p=P)
                    [:, :, mp * P:(mp + 2) * P])
                wk = pool.tile([P, KC, 2 * P], F32R, tag="wst", bufs=3)
                nc.sync.dma_start(
                    wk[:], Wk.rearrange("(c p) m -> p c m", p=P)
                    [:, :, mp * P:(mp + 2) * P])
                for m in (mp, mp + 1):
                    mo = (m - mp) * P
                    # Q projection chunk m
                    pq = ps.tile([P, T], F32, tag="mm", bufs=2)
                    for k in range(KC):
                        nc.tensor.matmul(pq[:], wq[:, k, mo:mo + P],
                                         q_src[:, k, :],
                                         start=(k == 0), stop=(k == KC - 1))
                    qt = pool.tile([P, T], F32R, tag="qt", bufs=2)
                    nc.scalar.activation(qt[:], pq[:], AF.Identity,
                                         bias=vec[:, qb_col + m:qb_col + m + 1])
                    if dbg and dbg_prefix == "s":
                        nc.sync.dma_start(dbg_d["d_qT"][m * P:(m + 1) * P, :],
                                          qt.bitcast(F32)[:])
                    # K projection chunk m
                    kt = pool.tile([P, S], F32R, tag="kt", bufs=2)
                    for n in range(2):
                        pk = ps.tile([P, T], F32, tag="mm", bufs=2)
                        for k in range(KC):
                            nc.tensor.matmul(pk[:], wk[:, k, mo:mo + P],
                                             x_kv[:, k, n * T:(n + 1) * T],
                                             start=(k == 0), stop=(k == KC - 1))
                        nc.scalar.activation(
                            kt[:, n * T:(n + 1) * T], pk[:], AF.Identity,
                            bias=vec[:, kb_col + m:kb_col + m + 1])
                    if dbg and dbg_prefix == "s":
                        nc.sync.dma_start(dbg_d["d_kT"][m * P:(m + 1) * P, :],
                                          kt.bitcast(F32)[:])
                    # the two heads of chunk m
                    for h2 in (1, 0):
                        h = HPC * m + h2
                        hb = h2 * D
                        psAv = ps.tile([P, T], F32, tag="av", bufs=2)
                        for i in range(KC):
                            pss = ps.tile([P, T], F32, tag="ss", bufs=2)
                            nc.tensor.matmul(pss[:],
                                             kt[hb:hb + D, i * P:(i + 1) * P],
                                             qt[hb:hb + D, :],
                                             start=True, stop=True)
                            et = pool.tile([P, T], F32R, tag="exp", bufs=3)
                            nc.scalar.activation(
                                et[:], pss[:], AF.Exp,
                                bias=vec[:, ab_col + i:ab_col + i + 1],
                                scale=1.0 / (SCALE * SCALE))
                            nc.tensor.matmul(psAv[0:D + 1, :],
                                             vt[:, i, h, :], et[:],
                                             start=(i == 0), stop=(i == KC - 1))
                        # reciprocal of denominator row (aligned at base D=64)
                        rden = pool.tile([P, T], F32R, tag="rden", bufs=1)
                        nc.vector.reciprocal(rden[D:D + 1, :], psAv[D:D + 1, :])
                        psB = ps.tile([P, T], F32, tag="av", bufs=2)
                        nc.tensor.matmul(psB[:], ones2[D:D + 1, :],
                                         rden[D:D + 1, :], start=True,
                                         stop=True)
                        rb = pool.tile([D, T], F32, tag="rb", bufs=2)
                        nc.vector.tensor_copy(rb[:], psB[0:D, :])
                        if h2 == 0:
                            nc.vector.tensor_tensor(at[0:D, m, :], psAv[0:D, :],
                                                    rb[:], op=OP.mult)
                        else:
                            atmp = pool.tile([D, T], F32R, tag="atmp", bufs=2)
                            nc.vector.tensor_tensor(atmp[:], psAv[0:D, :],
                                                    rb[:], op=OP.mult)
                            nc.sync.dma_start(at[D:P, m, :], atmp[:])
            if dbg and dbg_prefix == "s":
                nc.sync.dma_start(
                    dbg_d["d_at"].rearrange("(c p) t -> p c t", p=P),
                    at.bitcast(F32)[:])

            # out projection + bias' + residual (original xq)
            sa = glob.tile([P, KC, T], F32R, tag="res")
            for mp in range(0, KC, 2):
                if mp in wo_tiles:
                    wo = wo_tiles[mp]
                else:
                    wo = pool.tile([P, KC, 2 * P], F32R, tag="wst", bufs=3)
                    nc.sync.dma_start(
                        wo[:], Wo.rearrange("(c p) m -> p c m", p=P)
                        [:, :, mp * P:(mp + 2) * P])
                for m in (mp, mp + 1):
                    mo = (m - mp) * P
                    po = ps.tile([P, T], F32, tag="mm", bufs=2)
                    for k in range(KC):
                        nc.tensor.matmul(po[:], wo[:, k, mo:mo + P],
                                         at[:, k, :],
                                         start=(k == 0), stop=(k == KC - 1))
                    nc.vector.scalar_tensor_tensor(
                        sa[:, m, :], po[:], vec[:, ob_col + m:ob_col + m + 1],
                        xq.bitcast(F32)[:, m, :], op0=OP.add, op1=OP.add)
                    if ln_acc is not None and m > 0:
                        ln_sums_chunk(pool, ln_acc, sa[:, m - 1, :], m - 1)
            if ln_acc is not None:
                ln_sums_chunk(pool, ln_acc, sa[:, KC - 1, :], KC - 1)
            return sa

        # ====== self attention + LN1 + cross attention + LN2 (one pool) =====
        with tc.tile_pool(name="attn", bufs=1) as pool:
            xk = load_xfull(pool, xk_d)

            def _load_xq():
                nc.sync.dma_start(
                    xq[:], xq_d.rearrange("(c p) t -> p c t", p=P))

            acc1 = ln_sums_start()
            sa = attention(pool, xq, xk, w_d["sWq"], w_d["sWk"], w_d["sWv"],
                           w_d["sWo"], C_SBQ, C_SBK, C_SBO, C_SAB,
                           dbg_prefix="s", post_v_hook=_load_xq, ln_acc=acc1)
            if dbg:
                nc.sync.dma_start(
                    dbg_d["d_sa"].rearrange("(c p) t -> p c t", p=P),
                    sa.bitcast(F32)[:])
            snn = ln_finish(pool, acc1, sa, C_G, C_B, dbg_name="d_snn")
            xc = load_xfull(pool, xc_d)
            acc2 = ln_sums_start()
            ca = attention(pool, snn, xc, w_d["cWq"], w_d["cWk"], w_d["cWv"],
                           w_d["cWo"], C_CBQ, C_CBK, C_CBO, C_CAB,
                           ln_acc=acc2)

        # ================= FFN (LN2 inside, weights prefetched) ============
        with tc.tile_pool(name="ffn", bufs=1) as pool:
            w1r = w1_d.rearrange("(c p) m -> p c m", p=P)
            w2r = w2_d.rearrange("(c p) m -> p c m", p=P)
            w1_tiles = {}
            w1f = pool.tile([P, KC, P], F32R, tag="w1f", bufs=1)
            nc.sync.dma_start(w1f[:], w1r[:, :, 0:P])
            w1_tiles["f"] = w1f
            w1 = pool.tile([P, KC, 3 * P], F32R, tag="wst", bufs=2)
            nc.sync.dma_start(w1[:], w1r[:, :, P:4 * P])
            w1_tiles[0] = w1
            w1 = pool.tile([P, KC, 4 * P], F32R, tag="wst", bufs=2)
            nc.sync.dma_start(w1[:], w1r[:, :, 4 * P:8 * P])
            w1_tiles[4] = w1
            w2_tiles = {}
            for m0 in (0, 1):
                w2 = pool.tile([P, FC, P], F32R, tag="w2st", bufs=2)
                nc.sync.dma_start(w2[:], w2r[:, :, m0 * P:(m0 + 1) * P])
                w2_tiles[m0] = w2

            mb = pool.tile([P, T], F32, tag="lnb", bufs=2)
            ab = pool.tile([P, T], F32, tag="lnb", bufs=2)
            hT = ln_finish(pool, acc2, ca, C_G, C_B, dbg_name="d_h",
                           fused_copies=(mb, ab))

            # FFN1 consumes pre-LN ca directly; the LN correction commutes
            # through the contraction: u = relu((W1^T ca - colsum(W1) mean)
            # * rstd + b1)
            ut = pool.tile([P, FC, T], F32R, tag="ut")
            for mp in range(0, FC, 4):
                if mp in w1_tiles:
                    w1 = w1_tiles[mp]
                    moff = P if mp == 0 else 0
                elif mp == 4:
                    w1 = w1_tiles[4]
                    moff = -4 * P
                else:
                    w1 = pool.tile([P, KC, 4 * P], F32R, tag="wst", bufs=2)
                    nc.sync.dma_start(w1[:], w1r[:, :, mp * P:(mp + 4) * P])
                    moff = 0
                for m in range(mp, mp + 4):
                    if mp == 0 and m == 0:
                        w1u, mo = w1_tiles["f"], 0
                    elif mp == 0:
                        w1u, mo = w1, (m - 1) * P
                    else:
                        w1u, mo = w1, (m - mp) * P + moff
                    pu = ps.tile([P, T], F32, tag="mm", bufs=2)
                    for k in range(KC):
                        nc.tensor.matmul(pu[:], w1u[:, k, mo:mo + P],
                                         ca[:, k, :],
                                         start=(k == 0), stop=(k == KC - 1))
                    t1 = pool.tile([P, T], F32, tag="rb", bufs=2)
                    nc.vector.scalar_tensor_tensor(
                        t1[:], mb[:], vec[:, C_NW1 + m:C_NW1 + m + 1], pu[:],
                        op0=OP.mult, op1=OP.add)
                    nc.vector.tensor_tensor(t1[:], t1[:], ab[:], op=OP.mult)
                    nc.scalar.activation(ut[:, m, :], t1[:], AF.Relu,
                                         bias=vec[:, C_B1 + m:C_B1 + m + 1])
            if dbg:
                nc.sync.dma_start(
                    dbg_d["d_u"].rearrange("(c p) t -> p c t", p=P),
                    ut.bitcast(F32)[:])

            ff = glob.tile([P, KC, T], F32R, tag="res")
            acc3 = ln_sums_start()
            for m in range(KC):
                if m in w2_tiles:
                    w2 = w2_tiles[m]
                else:
                    w2 = pool.tile([P, FC, P], F32R, tag="w2st", bufs=2)
                    nc.sync.dma_start(w2[:], w2r[:, :, m * P:(m + 1) * P])
                pf = ps.tile([P, T], F32, tag="mm", bufs=2)
                for k in range(FC):
                    nc.tensor.matmul(pf[:], w2[:, k, :], ut[:, k, :],
                                     start=(k == 0), stop=(k == FC - 1))
                nc.vector.scalar_tensor_tensor(
                    ff[:, m, :], pf[:], vec[:, C_B2 + m:C_B2 + m + 1],
                    hT.bitcast(F32)[:, m, :], op0=OP.add, op1=OP.add)
                if m > 0:
                    ln_sums_chunk(pool, acc3, ff[:, m - 1, :], m - 1)
            ln_sums_chunk(pool, acc3, ff[:, KC - 1, :], KC - 1)

        with tc.tile_pool(name="ln3", bufs=1) as pool:
            ln_finish(pool, acc3, ff, C_G, C_B, out_dma=out_d)

    _legalize_waits(nc)
    return nc


_NC_CACHE = {}


def _get_nc(dbg=False):
    if dbg not in _NC_CACHE:
        _NC_CACHE[dbg] = _build(dbg)
    return _NC_CACHE[dbg]


def _pack_chunks(v):
    """[n*128] -> [128, n] with column m = v[m*128:(m+1)*128]."""
    n = v.shape[0] // P
    return np.ascontiguousarray(v.reshape(n, P).T)


def _make_in_maps(inputs):
    hs = np.asarray(inputs["hidden_states"], np.float32)
    chs = np.asarray(inputs["cross_hidden_states"], np.float32)
    smask = np.asarray(inputs["self_att_mask"], np.float32)
    cmask = np.asarray(inputs["cross_att_mask"], np.float32)

    f32 = lambda k: np.asarray(inputs[k], np.float32)
    bos = f32("sbo") + f32("sbv") @ f32("sWo")
    boc = f32("cbo") + f32("cbv") @ f32("cWo")

    base = {n: np.ascontiguousarray(f32(n)) for n in
            ["sWq", "sWk", "sWv", "sWo", "cWq", "cWk", "cWv", "cWo"]}
    base["W1"] = np.ascontiguousarray(f32("W1"))
    base["W2"] = np.ascontiguousarray(f32("W2"))
    base["ones2"] = np.ones((P, P), np.float32)

    vec = np.zeros((P, NVEC), np.float32)
    vec[:, C_SBQ:C_SBQ + 8] = _pack_chunks(f32("sbq"))
    vec[:, C_SBK:C_SBK + 8] = _pack_chunks(f32("sbk"))
    vec[:, C_SBO:C_SBO + 8] = _pack_chunks(bos)
    vec[:, C_CBQ:C_CBQ + 8] = _pack_chunks(f32("cbq"))
    vec[:, C_CBK:C_CBK + 8] = _pack_chunks(f32("cbk"))
    vec[:, C_CBO:C_CBO + 8] = _pack_chunks(boc)
    vec[:, C_G:C_G + 8] = _pack_chunks(f32("g"))
    vec[:, C_B:C_B + 8] = _pack_chunks(f32("b"))
    vec[:, C_B1:C_B1 + 32] = _pack_chunks(f32("b1"))
    vec[:, C_B2:C_B2 + 8] = _pack_chunks(f32("b2"))
    vec[:, C_NW1:C_NW1 + 32] = _pack_chunks(-f32("W1").sum(axis=0))
    vec[:, C_EPS] = EPS

    in_maps = []
    for c in range(8):
        b, qh = c // 2, c % 2
        qoff = qh * T
        m = dict(base)
        xkT = np.ascontiguousarray(hs[b].T)
        m["xkT"] = xkT
        m["xcT"] = np.ascontiguousarray(chs[b].T)
        m["xqT"] = np.ascontiguousarray(xkT[:, qoff:qoff + T])
        v = vec.copy()
        v[:, C_SAB:C_SAB + 8] = _pack_chunks((1.0 - smask[b]) * (-INF) / SCALE)
        v[:, C_CAB:C_CAB + 8] = _pack_chunks((1.0 - cmask[b]) * (-INF) / SCALE)
        m["vec"] = v
        in_maps.append(m)
    return in_maps


def _run(inputs, dbg=False):
    nc = _get_nc(dbg)
    in_maps = _make_in_maps(inputs)
    res = bass_utils.run_bass_kernel_spmd(nc, in_maps, core_ids=list(range(8)))
    return res.results


def kernel(**inputs) -> np.ndarray:
    results = _run(inputs, dbg=False)
    out = np.empty((B, S, H), np.float32)
    for c in range(8):
        b, qh = c // 2, c % 2
        out[b, qh * T:(qh + 1) * T, :] = results[c]["out"].T
    return out



# revision 43
# speedup vs baseline: 1.5063x; 1.5063x over previous
"""Trainium2 Bass kernel for nn_Block_30107720745811 (dense transformer block).

B=4, S=1024, H=1024, NH=16. 8 NeuronCores, zero-communication sharding:
core c computes batch b=c//2, query rows (c%2)*512:(c%2)*512+512.

All big GEMMs run as fp8e4m3 DoubleRow matmuls (0.5 cycles/row with a packed
256-deep contraction => 4x the fp32r PE rate). Activations live feature-major
[feature, token]; fp8 operands use the DoubleRow fold layout [128, 2, N]
(features 256c+128f+p at [p, f, n]). Q/K projections emit a d-folded layout
(head h=4a+b dim 32g+p at kf[32b+p, a, g, t]) via host-permuted weight
columns, so the per-head scores GEMM is also DoubleRow (K_p=32). The softmax
exp converts straight to fp8 folded tiles consumed by the att@V DoubleRow.
Residual-path tensors (xq, ca, h, ff) stay f32 for precision; the softmax
denominator rides att@V as a ones column of V.
"""
import numpy as np
import ml_dtypes
import concourse.bass as bass
import concourse.tile as tile
from concourse import mybir
from concourse import bass_utils
from concourse.alu_op_type import AluOpType as OP

AF = mybir.ActivationFunctionType
F32 = mybir.dt.float32
F32R = mybir.dt.float32r
F8 = mybir.dt.float8e4
DR = mybir.MatmulPerfMode.DoubleRow
FP8 = ml_dtypes.float8_e4m3

B, S, H, NH = 4, 1024, 1024, 16
D = H // NH          # 64
P = 128
T = 512              # query tokens per core
KC = H // P          # 8 feature chunks
C2 = 4               # 256-wide fold chunks
NS = 256             # V-proj output slice (4 heads)
VP = 80              # padded per-head vt row (D+1 used, 16B-aligned pitch)
INF = 1e10
EPS = 1e-5
SCALE = 8.0

# vec tensor column map
C_SBQ, C_SBK, C_SBO = 0, 8, 16
C_CBQ, C_CBK, C_CBO = 24, 32, 40
C_SAB, C_CAB = 48, 56
C_G, C_B = 64, 72
C_B1, C_B2, C_EPS = 80, 112, 120
C_CQS = 121
NVEC = 129

MAX_WAITS = 1


def _legalize_waits(nc, max_waits=MAX_WAITS):
    """Split >max_waits semaphore waits into preceding same-engine NOPs
    (this walrus build allows only one sync wait per instruction)."""
    n_split = 0
    for f in nc.m.functions:
        for blk in f.blocks:
            out = []
            for ins in blk.instructions:
                si = getattr(ins, "sync_info", None)
                if si is not None and si.on_wait and len(si.on_wait) > max_waits:
                    waits = list(si.on_wait)
                    extra, keep = waits[:-max_waits], waits[-max_waits:]
                    for j in range(0, len(extra), max_waits):
                        out.append(mybir.InstNoOp(
                            name=f"{ins.name}-lw{j}",
                            engine=ins.engine,
                            sync_info=mybir.SyncInfo(
                                on_wait=extra[j:j + max_waits], on_update=[]),
                            bass_nofuse=True,
                        ))
                    ins.sync_info = mybir.SyncInfo(
                        on_wait=keep, on_update=list(si.on_update))
                    n_split += 1
                out.append(ins)
            blk.instructions = out
    return n_split


def _build(dbg=False, masked=False, gbtriv=True):
    nc = bass.Bass("TRN2", target_bir_lowering=False, debug=False,
                   dynamic_dma_scratch_size=8192)

    def din(name, shape, dt=F32):
        return nc.dram_tensor(name, shape, dt, kind="ExternalInput").ap()

    xk_d = din("xkT", [H, S])            # self hidden, transposed
    xc_d = din("xcT", [H, S])            # cross hidden, transposed
    xq_d = din("xqT", [H, T])            # query cols of self hidden
    w8 = {}
    for p_ in ("s", "c"):
        w8[p_ + "qf"] = din(p_ + "WqF", [P, C2, 4, 2, 2, P], F8)
        w8[p_ + "kf"] = din(p_ + "WkF", [P, C2, 4, 2, 2, P], F8)
        w8[p_ + "v"] = din(p_ + "Wv8", [P, C2, 2, H], F8)
        w8[p_ + "o"] = din(p_ + "Wo8", [P, C2, 2, KC, P], F8)
    w18_d = din("W18", [P, 4, 2, C2, 2, 8, P], F8)  # m-blk, hi/lo outermost
    w28_d = din("W28", [P, KC, 2, 16, 2, P], F8)    # out-chunk m, hi/lo
    vec_d = din("vec", [P, NVEC], F32)
    ones2_d = din("ones2", [P, P], F32R)
    out_d = nc.dram_tensor("out", [H, T], F32, kind="ExternalOutput").ap()
    dbg_d = {}
    if dbg:
        for n, shape, dt in [("d_kf", [P, C2, 2, S], F8),
                             ("d_qf", [P, C2, 2, T], F8),
                             ("d_vt", [P, C2, 2, NH, VP], F8),
                             ("d_at", [P, C2, 2, T], F8),
                             ("d_sa", [P, C2, 2, T], F8),
                             ("d_snn", [P, C2, 2, T], F8),
                             ("d_h", [H, T], F32),
                             ("d_u", [P, 16, 2, T], F8)]:
            dbg_d[n] = nc.dram_tensor(n, shape, dt, kind="ExternalOutput").ap()

    with (
        tile.TileContext(nc) as tc,
        nc.allow_low_precision(reason="fp8 matmuls, f32 residual path"),
        tc.tile_pool(name="glob", bufs=1) as glob,
        tc.tile_pool(name="ps", bufs=1, space="PSUM") as ps,
    ):
        # ---- constants / persistent tiles ----
        vec = glob.tile([P, NVEC], F32, tag="vec")
        nc.sync.dma_start(vec[:], vec_d[:])
        ones2 = glob.tile([P, P], F32R, tag="ones2")
        nc.sync.dma_start(ones2[:], ones2_d[:])
        ones8 = glob.tile([P, 2, 16], F8, tag="ones8")
        nc.vector.memset(ones8[:], 1.0)
        h_t = glob.tile([P, KC, T], F32, tag="h")       # LN2 out (residual)
        hf8 = glob.tile([P, C2, 2, T], F8, tag="hf8")   # LN2 out fp8 (FFN1)
        hl8 = glob.tile([P, C2, 2, T], F8, tag="hl8")   # fp8 residual of h
        hs8 = glob.tile([P, C2, 2, T], F8, tag="hs8")   # h/64 for W1lo pass

        def ps_mm():
            return ps.tile([P, T], F32, tag="mm", bufs=2, name="psmm")

        def ps_sc():
            return ps.tile([P, 2, T], F32, tag="sc", bufs=2, name="pssc")

        def ps_av():
            return ps.tile([P, T], F32, tag="av", bufs=2, name="psav")

        # ============================ attention ============================
        def proj_v_units(pool, kv8, wv_t, on_act=False):
            """V projection units -> vt [P, i2, fold, head, 0:D] fp8."""
            vt = pool.tile([P, C2, 2, NH, VP], F8, tag="vt", bufs=2)
            nc.vector.memset(vt[:, :, :, :, D:D + 1], 1.0)

            def unit(i, n):
                pv = ps_mm()
                for c2 in range(C2):
                    nc.tensor.matmul(
                        pv[:, 0:NS], kv8[:, c2, :, i * P:(i + 1) * P],
                        wv_t[:, c2, :, n * NS:(n + 1) * NS],
                        start=(c2 == 0), stop=(c2 == C2 - 1), perf_mode=DR)
                dst = vt[:, i // 2, i % 2, 4 * n:4 * (n + 1), 0:D]
                src_ = pv[:, 0:NS].rearrange("p (h d) -> p h d", d=D)
                if on_act:
                    nc.scalar.copy(dst, src_)
                else:
                    nc.vector.tensor_copy(dst, src_)

            units = [(unit, i, n) for i in range(KC) for n in range(4)]
            return vt, units

        def proj_v(pool, kv8, wv_t, on_act=False):
            vt, units = proj_v_units(pool, kv8, wv_t, on_act)
            for u, i, n in units:
                u(i, n)
            return vt

        def proj_kf_units(pool, kv8, wk_t, kb, on_act):
            """K projection units, d-folded output kf[32b+p, a, g, s]."""
            kf = pool.tile([P, C2, 2, S], F8, tag="kf", bufs=2)

            def unit(a, g, n2):
                pk = ps_mm()
                for c2 in range(C2):
                    nc.tensor.matmul(
                        pk[:], wk_t[:, c2, a, g, :, :],
                        kv8[:, c2, :, n2 * T:(n2 + 1) * T],
                        start=(c2 == 0), stop=(c2 == C2 - 1), perf_mode=DR)
                col = kb + 2 * a + g
                dst = kf[:, a, g, n2 * T:(n2 + 1) * T]
                if on_act:
                    nc.scalar.activation(dst, pk[:], AF.Identity,
                                         bias=vec[:, col:col + 1])
                else:
                    nc.vector.tensor_scalar(dst, pk[:], vec[:, col:col + 1],
                                            None, op0=OP.add)

            units = [(unit, a, g, n2) for a in range(4) for g in range(2)
                     for n2 in range(2)]
            return kf, units

        def proj_kf(pool, kv8, wk_t, kb, on_act):
            kf, units = proj_kf_units(pool, kv8, wk_t, kb, on_act)
            for u, a, g, n2 in units:
                u(a, g, n2)
            return kf

        def proj_qf(pool, q8, wq_t, qb, on_act=True):
            qf = pool.tile([P, C2, 2, T], F8, tag="qf")
            for a in range(4):
                for g in range(2):
                    pq = ps_mm()
                    for c2 in range(C2):
                        nc.tensor.matmul(
                            pq[:], wq_t[:, c2, a, g, :, :], q8[:, c2, :, :],
                            start=(c2 == 0), stop=(c2 == C2 - 1), perf_mode=DR)
                    col = qb + 2 * a + g
                    if on_act:
                        nc.scalar.activation(qf[:, a, g, :], pq[:],
                                             AF.Identity,
                                             bias=vec[:, col:col + 1])
                    else:
                        nc.vector.tensor_scalar(qf[:, a, g, :], pq[:],
                                                vec[:, col:col + 1], None,
                                                op0=OP.add)
            return qf

        def scores_av(pool, kf, qf, vt, ab, hooks=None):
            """Per-head softmax(scores)@V -> at8 [P, k2, fold, T] fp8."""
            at8 = pool.tile([P, C2, 2, T], F8, tag="at8")
            hidx = 0
            for m in range(KC):
                for h2 in (1, 0):
                    if hooks and hidx in hooks:
                        hooks[hidx]()
                    hidx += 1
                    h = 2 * m + h2
                    a, b_ = h // 4, h % 4
                    psAv = ps_av()
                    for i2 in range(C2):
                        sc = ps_sc()
                        for j in (0, 1):
                            i = 2 * i2 + j
                            nc.tensor.matmul(
                                sc[:, j, :],
                                kf[32 * b_:32 * b_ + 32, a, :,
                                   i * P:(i + 1) * P],
                                qf[32 * b_:32 * b_ + 32, a, :, :],
                                start=True, stop=True, perf_mode=DR,
                                tile_position=(32 * b_, 0))
                        et = pool.tile([P, 2, T], F8, tag="et", bufs=4)
                        if masked:
                            for j in (0, 1):
                                col = ab + 2 * i2 + j
                                nc.scalar.activation(
                                    et[:, j, :], sc[:, j, :], AF.Exp,
                                    bias=vec[:, col:col + 1],
                                    scale=1.0 / (SCALE * SCALE))
                        else:
                            nc.scalar.activation(et[:], sc[:], AF.Exp,
                                                 scale=1.0 / (SCALE * SCALE))
                        nc.tensor.matmul(psAv[0:D + 1, :],
                                         vt[:, i2, :, h, 0:D + 1], et[:],
                                         start=(i2 == 0), stop=(i2 == C2 - 1),
                                         perf_mode=DR)
                    rden = pool.tile([1, T], F32R, tag="rden", bufs=2)
                    nc.vector.reciprocal(rden[:], psAv[D:D + 1, :])
                    psB = ps_av()
                    nc.tensor.matmul(psB[0:D, :], ones2[0:1, 0:D], rden[:],
                                     start=True, stop=True)
                    rb = pool.tile([D, T], F32, tag="rb", bufs=2)
                    nc.vector.tensor_copy(rb[:], psB[0:D, :])
                    if h2 == 0:
                        nc.vector.tensor_tensor(
                            at8[0:D, m // 2, m % 2, :], psAv[0:D, :],
                            rb[:], op=OP.mult)
                    else:
                        atmp = pool.tile([D, T], F8, tag="atmp", bufs=2)
                        nc.vector.tensor_tensor(atmp[:], psAv[0:D, :],
                                                rb[:], op=OP.mult)
                        nc.gpsimd.dma_start(at8[D:P, m // 2, m % 2, :],
                                            atmp[:])
            return at8

        def proj_o(pool, at8, wo_t, ob, resid, out_f8=None, out_f32=None,
                   ln_hook=None):
            """Out-proj + bias' + residual; optional per-chunk LN-sum hook."""
            for m in range(KC):
                po = ps_mm()
                for k2 in range(C2):
                    nc.tensor.matmul(po[:], wo_t[:, k2, :, m, :],
                                     at8[:, k2, :, :],
                                     start=(k2 == 0), stop=(k2 == C2 - 1),
                                     perf_mode=DR)
                dst = (out_f8[:, m // 2, m % 2, :] if out_f8 is not None
                       else out_f32[:, m, :])
                nc.vector.scalar_tensor_tensor(
                    dst, po[:], vec[:, ob + m:ob + m + 1], resid[:, m, :],
                    op0=OP.add, op1=OP.add)
                if ln_hook is not None:
                    ln_hook(m)

        # ======================== layernorm helpers ========================
        def ln_math(pool, psS, psQ):
            """[1,T] sums -> sc tile with rstd bcast [:,0,:], mean bcast
            [:,1,:]."""
            mean = pool.tile([1, T], F32, tag="lnv", bufs=3)
            nc.scalar.mul(mean[:], psS, 1.0 / H)
            ex2 = pool.tile([1, T], F32, tag="lnv", bufs=3)
            nc.scalar.mul(ex2[:], psQ, 1.0 / H)
            var = pool.tile([1, T], F32, tag="lnv", bufs=3)
            nc.vector.tensor_tensor(var[:], mean[:], mean[:], op=OP.mult)
            nc.vector.tensor_tensor(var[:], ex2[:], var[:], op=OP.subtract)
            lv = pool.tile([1, T], F32, tag="lnv", bufs=3)
            nc.scalar.activation(lv[:], var[:], AF.Ln,
                                 bias=vec[0:1, C_EPS:C_EPS + 1])
            rstd = pool.tile([1, T], F32R, tag="lnr", bufs=2)
            nc.scalar.activation(rstd[:], lv[:], AF.Exp, scale=-0.5)
            meanr = pool.tile([1, T], F32R, tag="lnr", bufs=2)
            nc.vector.tensor_copy(meanr[:], mean[:])
            scB = ps_sc()
            nc.tensor.matmul(scB[:, 0, :], ones2[0:1, :], rstd[:],
                             start=True, stop=True)
            nc.tensor.matmul(scB[:, 1, :], ones2[0:1, :], meanr[:],
                             start=True, stop=True)
            sbB = pool.tile([P, 2, T], F32, tag="sbB", bufs=1)
            nc.vector.tensor_copy(sbB[:], scB[:])
            return sbB

        def ln_stats_f8(pool, src8):
            """LN sums from an fp8-folded [P, C2, 2, T] tensor."""
            scS = ps_sc()
            for j in range(C2):
                nc.tensor.matmul(scS[0:1, 0, :], ones8[:, :, 0:1],
                                 src8[:, j, :, :], start=(j == 0),
                                 stop=(j == C2 - 1), perf_mode=DR,
                                 skip_group_check=True)
            for j in range(C2):
                sq8 = pool.tile([P, 2, T], F8, tag="sq8", bufs=2)
                nc.gpsimd.tensor_tensor(sq8[:], src8[:, j, :, :],
                                        src8[:, j, :, :], op=OP.mult)
                nc.tensor.matmul(scS[0:1, 1, :], ones8[:, :, 0:1], sq8[:],
                                 start=(j == 0), stop=(j == C2 - 1),
                                 perf_mode=DR, skip_group_check=True)
            return scS

        def ln_dst(pool, src_ap, sbB, m, dst_ap):
            """dst = g*(src - mean)*rstd + b for one [P, T] chunk."""
            e1 = nc.gpsimd if m % 2 == 0 else nc.vector
            e2 = nc.vector if m % 2 == 0 else nc.gpsimd
            t1 = pool.tile([P, T], F32, tag="t1", bufs=3)
            e1.tensor_tensor(t1[:], src_ap, sbB[:, 1, :], op=OP.subtract)
            if gbtriv:
                e2.tensor_tensor(dst_ap, t1[:], sbB[:, 0, :], op=OP.mult)
            else:
                e2.tensor_tensor(t1[:], t1[:], sbB[:, 0, :], op=OP.mult)
                e1.tensor_scalar(dst_ap, t1[:], vec[:, C_G + m:C_G + m + 1],
                                 vec[:, C_B + m:C_B + m + 1], op0=OP.mult,
                                 op1=OP.add)

        # =========================== main program ==========================
        with tc.tile_pool(name="attn", bufs=1) as pool:
            def load_w(tag, shape, dram):
                t = pool.tile(shape, F8, tag=tag, bufs=1, name=tag)
                nc.sync.dma_start(t[:], dram[:])
                return t

            # V-proj weights first (first consumer of xk8), then stream xk
            wv_s = load_w("wv", [P, C2, 2, H], w8["sv"])
            xk8 = pool.tile([P, C2, 2, S], F8, tag="xk8")
            xkr = xk_d.rearrange("(c p) t -> p c t", p=P)
            for blk in range(8):
                stg = pool.tile([P, KC, P], F32, tag="stg", bufs=2)
                nc.sync.dma_start(stg[:], xkr[:, :, blk * P:(blk + 1) * P])
                nc.gpsimd.tensor_copy(
                    xk8.rearrange("p c f (b t) -> p c f b t", t=P)
                    [:, :, :, blk, :],
                    stg.rearrange("p (c f) t -> p c f t", f=2)[:])
            # K weights + query slice after the hidden stream
            wk_s = pool.tile([P, C2, 4, 2, 2, P], F8, tag="wk", bufs=1,
                             name="wk")
            nc.sync.dma_start(wk_s[:, :, 0:2, :, :, :],
                              w8["skf"][:, :, 0:2, :, :, :])
            xq = pool.tile([P, KC, T], F32, tag="xq")
            nc.sync.dma_start(xq[:], xq_d.rearrange("(c p) t -> p c t", p=P))
            xq8 = pool.tile([P, C2, 2, T], F8, tag="xq8")
            for c in range(KC):
                nc.gpsimd.tensor_copy(xq8[:, c // 2, c % 2, :], xq[:, c, :])
            wq_s = pool.tile([P, C2, 4, 2, 2, P], F8, tag="wq", bufs=1,
                             name="wq")
            nc.sync.dma_start(wq_s[:, :, 0:2, :, :, :],
                              w8["sqf"][:, :, 0:2, :, :, :])
            nc.sync.dma_start(wk_s[:, :, 2:4, :, :, :],
                              w8["skf"][:, :, 2:4, :, :, :])
            nc.sync.dma_start(wq_s[:, :, 2:4, :, :, :],
                              w8["sqf"][:, :, 2:4, :, :, :])
            wo_s = load_w("wo", [P, C2, 2, KC, P], w8["so"])

            # ---- cross hidden staging + fold (early; overlaps self attn) ----
            xc8 = pool.tile([P, C2, 2, S], F8, tag="xc8")
            xcr = xc_d.rearrange("(c p) t -> p c t", p=P)
            for j in range(KC):
                stg = pool.tile([P, S], F32, tag="stg", bufs=2)
                nc.sync.dma_start(stg[:], xcr[:, j, :])
                nc.gpsimd.tensor_copy(xc8[:, j // 2, j % 2, :], stg[:])
            # cross weights stream while self attention computes
            wv_c = load_w("wv", [P, C2, 2, H], w8["cv"])
            wk_c = load_w("wk", [P, C2, 4, 2, 2, P], w8["ckf"])
            wq_c = load_w("wq", [P, C2, 4, 2, 2, P], w8["cqf"])
            wo_c = load_w("wo", [P, C2, 2, KC, P], w8["co"])

            vt = proj_v(pool, xk8, wv_s, on_act=True)
            kf = proj_kf(pool, xk8, wk_s, C_SBK, on_act=True)
            qf = proj_qf(pool, xq8, wq_s, C_SBQ)
            vt_c, v_units = proj_v_units(pool, xc8, wv_c)
            kf_c, k_units = proj_kf_units(pool, xc8, wk_c, C_CBK,
                                          on_act=False)
            work = ([("v",) + u[1:] for u in v_units]
                    + [("k",) + u[1:] for u in k_units])
            v_unit, k_unit = v_units[0][0], k_units[0][0]
            hooks = {}
            for hi in range(2, 14):
                lo = (hi - 2) * 4
                chunk = work[lo:lo + 4]

                def _mk(chunk):
                    def _h():
                        for w in chunk:
                            if w[0] == "v":
                                v_unit(w[1], w[2])
                            else:
                                k_unit(w[1], w[2], w[3])
                    return _h

                hooks[hi] = _mk(chunk)
            at8 = scores_av(pool, kf, qf, vt, C_SAB, hooks=hooks)
            if dbg:
                nc.sync.dma_start(dbg_d["d_kf"][:], kf[:])
                nc.sync.dma_start(dbg_d["d_qf"][:], qf[:])
                nc.sync.dma_start(dbg_d["d_vt"][:], vt[:])
                nc.sync.dma_start(dbg_d["d_at"][:], at8[:])
            sa8 = pool.tile([P, C2, 2, T], F8, tag="sa8")
            acc1 = {}

            def ln1_hook(m):
                if m % 2 == 0:
                    return
                j = m // 2
                if "scS" not in acc1:
                    acc1["scS"] = ps_sc()
                scS = acc1["scS"]
                nc.tensor.matmul(scS[0:1, 0, :], ones8[:, :, 0:1],
                                 sa8[:, j, :, :], start=(j == 0),
                                 stop=(j == C2 - 1), perf_mode=DR,
                                 skip_group_check=True)
                sq8 = pool.tile([P, 2, T], F8, tag="sq8", bufs=2)
                nc.gpsimd.tensor_tensor(sq8[:], sa8[:, j, :, :],
                                        sa8[:, j, :, :], op=OP.mult)
                nc.tensor.matmul(scS[0:1, 1, :], ones8[:, :, 0:1], sq8[:],
                                 start=(j == 0), stop=(j == C2 - 1),
                                 perf_mode=DR, skip_group_check=True)

            proj_o(pool, at8, wo_s, C_SBO, xq, out_f8=sa8, ln_hook=ln1_hook)
            if dbg:
                nc.sync.dma_start(dbg_d["d_sa"][:], sa8[:])

            # ---- LN1 stats + fused cross-Q ----
            # q_c = rstd * (sa@Wq' - mean*colsum(Wq')) + bias'; the Wq'
            # matmuls consume sa8 directly, overlapping the LN1 math.
            scS1 = acc1["scS"]
            qf_c = pool.tile([P, C2, 2, T], F8, tag="qf")
            scB1 = None
            for a in range(4):
                for g in range(2):
                    i_ = 2 * a + g
                    pq = ps_mm()
                    for c2 in range(C2):
                        nc.tensor.matmul(
                            pq[:], wq_c[:, c2, a, g, :, :], sa8[:, c2, :, :],
                            start=(c2 == 0), stop=(c2 == C2 - 1), perf_mode=DR)
                    if scB1 is None:
                        scB1 = ln_math(pool, scS1[0:1, 0, :],
                                       scS1[0:1, 1, :])
                    t1 = pool.tile([P, T], F32, tag="t1", bufs=3)
                    nc.vector.scalar_tensor_tensor(
                        t1[:], scB1[:, 1, :],
                        vec[:, C_CQS + i_:C_CQS + i_ + 1], pq[:],
                        op0=OP.mult, op1=OP.add)
                    nc.vector.tensor_tensor(t1[:], t1[:], scB1[:, 0, :],
                                            op=OP.mult)
                    nc.scalar.activation(qf_c[:, a, g, :], t1[:], AF.Identity,
                                         bias=vec[:, C_CBQ + i_:
                                                  C_CBQ + i_ + 1])
            at8_c = scores_av(pool, kf_c, qf_c, vt_c, C_CAB)
            ca = pool.tile([P, KC, T], F32R, tag="ca")
            acc2 = {}

            def ln2_hook(m):
                # ride LN2 sums on ca chunks as they complete
                if "scS" not in acc2:
                    acc2["scS"] = ps_sc()
                scS = acc2["scS"]
                nc.tensor.matmul(scS[0:1, 0, :], ones2[:, 0:1], ca[:, m, :],
                                 start=(m == 0), stop=(m == KC - 1),
                                 skip_group_check=True)
                sq = pool.tile([P, T], F32R, tag="sq", bufs=2)
                nc.vector.tensor_tensor(sq[:], ca.bitcast(F32)[:, m, :],
                                        ca.bitcast(F32)[:, m, :], op=OP.mult)
                nc.tensor.matmul(scS[0:1, 1, :], ones2[:, 0:1], sq[:],
                                 start=(m == 0), stop=(m == KC - 1),
                                 skip_group_check=True)

            proj_o(pool, at8_c, wo_c, C_CBO, xq, out_f32=ca,
                   ln_hook=ln2_hook)

            # ---- LN2 (on ca) -> h (f32) + hf8 ----
            scS2 = acc2["scS"]
            scB2 = ln_math(pool, scS2[0:1, 0, :], scS2[0:1, 1, :])
            for m in range(KC):
                ln_dst(pool, ca.bitcast(F32)[:, m, :], scB2, m, h_t[:, m, :])
                nc.scalar.copy(hf8[:, m // 2, m % 2, :], h_t[:, m, :])
                nc.gpsimd.tensor_tensor(hl8[:, m // 2, m % 2, :],
                                        h_t[:, m, :],
                                        hf8[:, m // 2, m % 2, :],
                                        op=OP.subtract)
                nc.scalar.mul(hs8[:, m // 2, m % 2, :], h_t[:, m, :],
                              1.0 / 64.0)
            if dbg:
                nc.sync.dma_start(
                    dbg_d["d_h"].rearrange("(c p) t -> p c t", p=P), h_t[:])

        # ================= FFN (fp8 DoubleRow) + final LN ==================
        with tc.tile_pool(name="ffn", bufs=1) as pool:
            ut8 = pool.tile([P, 16, 2, T], F8, tag="ut8")
            us8 = pool.tile([P, 16, 2, T], F8, tag="us8")
            for m0 in range(0, 32, 8):
                w1t = pool.tile([P, 2, C2, 2, 8, P], F8, tag="w1t", bufs=2)
                nc.sync.dma_start(w1t[:], w18_d[:, m0 // 8, :, :, :, :, :])
                for m in range(m0, m0 + 8):
                    pu = ps_mm()
                    passes = [(0, hf8), (0, hl8), (1, hs8)]
                    for pi, (lo, hsrc) in enumerate(passes):
                        for c2 in range(C2):
                            nc.tensor.matmul(
                                pu[:], w1t[:, lo, c2, :, m - m0, :],
                                hsrc[:, c2, :, :],
                                start=(pi == 0 and c2 == 0),
                                stop=(pi == 2 and c2 == C2 - 1),
                                perf_mode=DR)
                    nc.vector.tensor_scalar(ut8[:, m // 2, m % 2, :], pu[:],
                                      vec[:, C_B1 + m:C_B1 + m + 1], 0.0,
                                      op0=OP.add, op1=OP.max)
                    nc.gpsimd.tensor_scalar(us8[:, m // 2, m % 2, :],
                                            ut8[:, m // 2, m % 2, :],
                                            1.0 / 64.0, None, op0=OP.mult)
            if dbg:
                nc.sync.dma_start(dbg_d["d_u"][:], ut8[:])

            ff = pool.tile([P, KC, T], F32R, tag="ff")
            acc3 = {}
            for m in range(KC):
                w2t = pool.tile([P, 2, 16, 2, P], F8, tag="w2t", bufs=3)
                nc.sync.dma_start(w2t[:], w28_d[:, m, :, :, :, :])
                pf = ps_mm()
                for lo, usrc in ((0, ut8), (1, us8)):
                    for k2 in range(16):
                        nc.tensor.matmul(pf[:], w2t[:, lo, k2, :, :],
                                         usrc[:, k2, :, :],
                                         start=(lo == 0 and k2 == 0),
                                         stop=(lo == 1 and k2 == 15),
                                         perf_mode=DR)
                nc.vector.scalar_tensor_tensor(
                    ff[:, m, :], pf[:], vec[:, C_B2 + m:C_B2 + m + 1],
                    h_t[:, m, :], op0=OP.add, op1=OP.add)
                if "scS" not in acc3:
                    acc3["scS"] = ps_sc()
                scS = acc3["scS"]
                nc.tensor.matmul(scS[0:1, 0, :], ones2[:, 0:1], ff[:, m, :],
                                 start=(m == 0), stop=(m == KC - 1),
                                 skip_group_check=True)
                sq = pool.tile([P, T], F32R, tag="sq", bufs=2)
                nc.gpsimd.tensor_tensor(sq[:], ff.bitcast(F32)[:, m, :],
                                        ff.bitcast(F32)[:, m, :], op=OP.mult)
                nc.tensor.matmul(scS[0:1, 1, :], ones2[:, 0:1], sq[:],
                                 start=(m == 0), stop=(m == KC - 1),
                                 skip_group_check=True)

            scS3 = acc3["scS"]
            scB3 = ln_math(pool, scS3[0:1, 0, :], scS3[0:1, 1, :])
            for m in range(KC):
                ob = pool.tile([P, T], F32, tag="ob", bufs=3)
                ln_dst(pool, ff.bitcast(F32)[:, m, :], scB3, m, ob[:])
                nc.sync.dma_start(out_d[m * P:(m + 1) * P, :], ob[:])

    _legalize_waits(nc)
    return nc


# revision 44
# speedup vs baseline: 1.5093x; 1.0020x over previous
"""Trainium2 Bass kernel for nn_Block_30107720745811 (dense transformer block).

B=4, S=1024, H=1024, NH=16. 8 NeuronCores, zero-communication sharding:
core c computes batch b=c//2, query rows (c%2)*512:(c%2)*512+512.

All big GEMMs run as fp8e4m3 DoubleRow matmuls (0.5 cycles/row with a packed
256-deep contraction => 4x the fp32r PE rate). Activations live feature-major
[feature, token]; fp8 operands use the DoubleRow fold layout [128, 2, N]
(features 256c+128f+p at [p, f, n]). Q/K projections emit a d-folded layout
(head h=4a+b dim 32g+p at kf[32b+p, a, g, t]) via host-permuted weight
columns, so the per-head scores GEMM is also DoubleRow (K_p=32). The softmax
exp converts straight to fp8 folded tiles consumed by the att@V DoubleRow.
Residual-path tensors (xq, ca, h, ff) stay f32 for precision; the softmax
denominator rides att@V as a ones column of V.
"""
import numpy as np
import ml_dtypes
import concourse.bass as bass
import concourse.tile as tile
from concourse import mybir
from concourse import bass_utils
from concourse.alu_op_type import AluOpType as OP

AF = mybir.ActivationFunctionType
F32 = mybir.dt.float32
F32R = mybir.dt.float32r
F8 = mybir.dt.float8e4
DR = mybir.MatmulPerfMode.DoubleRow
FP8 = ml_dtypes.float8_e4m3

B, S, H, NH = 4, 1024, 1024, 16
D = H // NH          # 64
P = 128
T = 512              # query tokens per core
KC = H // P          # 8 feature chunks
C2 = 4               # 256-wide fold chunks
NS = 256             # V-proj output slice (4 heads)
VP = 80              # padded per-head vt row (D+1 used, 16B-aligned pitch)
INF = 1e10
EPS = 1e-5
SCALE = 8.0

# vec tensor column map
C_SBQ, C_SBK, C_SBO = 0, 8, 16
C_CBQ, C_CBK, C_CBO = 24, 32, 40
C_SAB, C_CAB = 48, 56
C_G, C_B = 64, 72
C_B1, C_B2, C_EPS = 80, 112, 120
C_CQS = 121
NVEC = 129

MAX_WAITS = 1


def _legalize_waits(nc, max_waits=MAX_WAITS):
    """Split >max_waits semaphore waits into preceding same-engine NOPs
    (this walrus build allows only one sync wait per instruction)."""
    n_split = 0
    for f in nc.m.functions:
        for blk in f.blocks:
            out = []
            for ins in blk.instructions:
                si = getattr(ins, "sync_info", None)
                if si is not None and si.on_wait and len(si.on_wait) > max_waits:
                    waits = list(si.on_wait)
                    extra, keep = waits[:-max_waits], waits[-max_waits:]
                    for j in range(0, len(extra), max_waits):
                        out.append(mybir.InstNoOp(
                            name=f"{ins.name}-lw{j}",
                            engine=ins.engine,
                            sync_info=mybir.SyncInfo(
                                on_wait=extra[j:j + max_waits], on_update=[]),
                            bass_nofuse=True,
                        ))
                    ins.sync_info = mybir.SyncInfo(
                        on_wait=keep, on_update=list(si.on_update))
                    n_split += 1
                out.append(ins)
            blk.instructions = out
    return n_split


def _build(dbg=False, masked=False, gbtriv=True):
    nc = bass.Bass("TRN2", target_bir_lowering=False, debug=False,
                   dynamic_dma_scratch_size=8192)

    def din(name, shape, dt=F32):
        return nc.dram_tensor(name, shape, dt, kind="ExternalInput").ap()

    xk_d = din("xkT", [H, S])            # self hidden, transposed
    xc_d = din("xcT", [H, S])            # cross hidden, transposed
    xq_d = din("xqT", [H, T])            # query cols of self hidden
    w8 = {}
    for p_ in ("s", "c"):
        w8[p_ + "qf"] = din(p_ + "WqF", [P, C2, 4, 2, 2, P], F8)
        w8[p_ + "kf"] = din(p_ + "WkF", [P, C2, 4, 2, 2, P], F8)
        w8[p_ + "v"] = din(p_ + "Wv8", [P, C2, 2, H], F8)
        w8[p_ + "o"] = din(p_ + "Wo8", [P, C2, 2, KC, P], F8)
    w18_d = din("W18", [P, 4, 2, C2, 2, 8, P], F8)  # m-blk, hi/lo outermost
    w28_d = din("W28", [P, KC, 2, 16, 2, P], F8)    # out-chunk m, hi/lo
    vec_d = din("vec", [P, NVEC], F32)
    ones2_d = din("ones2", [P, P], F32R)
    out_d = nc.dram_tensor("out", [H, T], F32, kind="ExternalOutput").ap()
    dbg_d = {}
    if dbg:
        for n, shape, dt in [("d_kf", [P, C2, 2, S], F8),
                             ("d_qf", [P, C2, 2, T], F8),
                             ("d_vt", [P, C2, 2, NH, VP], F8),
                             ("d_at", [P, C2, 2, T], F8),
                             ("d_sa", [P, C2, 2, T], F8),
                             ("d_snn", [P, C2, 2, T], F8),
                             ("d_h", [H, T], F32),
                             ("d_u", [P, 16, 2, T], F8)]:
            dbg_d[n] = nc.dram_tensor(n, shape, dt, kind="ExternalOutput").ap()

    with (
        tile.TileContext(nc) as tc,
        nc.allow_low_precision(reason="fp8 matmuls, f32 residual path"),
        tc.tile_pool(name="glob", bufs=1) as glob,
        tc.tile_pool(name="ps", bufs=1, space="PSUM") as ps,
    ):
        # ---- constants / persistent tiles ----
        vec = glob.tile([P, NVEC], F32, tag="vec")
        nc.sync.dma_start(vec[:], vec_d[:])
        ones2 = glob.tile([P, P], F32R, tag="ones2")
        nc.sync.dma_start(ones2[:], ones2_d[:])
        ones8 = glob.tile([P, 2, 16], F8, tag="ones8")
        nc.vector.memset(ones8[:], 1.0)
        h_t = glob.tile([P, KC, T], F32, tag="h")       # LN2 out (residual)
        hf8 = glob.tile([P, C2, 2, T], F8, tag="hf8")   # LN2 out fp8 (FFN1)
        hl8 = glob.tile([P, C2, 2, T], F8, tag="hl8")   # fp8 residual of h
        hs8 = glob.tile([P, C2, 2, T], F8, tag="hs8")   # h/64 for W1lo pass

        def ps_mm():
            return ps.tile([P, T], F32, tag="mm", bufs=2, name="psmm")

        def ps_sc():
            return ps.tile([P, 2, T], F32, tag="sc", bufs=2, name="pssc")

        def ps_av():
            return ps.tile([P, T], F32, tag="av", bufs=2, name="psav")

        # ============================ attention ============================
        def proj_v_units(pool, kv8, wv_t, on_act=False):
            """V projection units -> vt [P, i2, fold, head, 0:D] fp8."""
            vt = pool.tile([P, C2, 2, NH, VP], F8, tag="vt", bufs=2)
            nc.vector.memset(vt[:, :, :, :, D:D + 1], 1.0)

            def unit(i, n):
                pv = ps_mm()
                for c2 in range(C2):
                    nc.tensor.matmul(
                        pv[:, 0:NS], kv8[:, c2, :, i * P:(i + 1) * P],
                        wv_t[:, c2, :, n * NS:(n + 1) * NS],
                        start=(c2 == 0), stop=(c2 == C2 - 1), perf_mode=DR)
                dst = vt[:, i // 2, i % 2, 4 * n:4 * (n + 1), 0:D]
                src_ = pv[:, 0:NS].rearrange("p (h d) -> p h d", d=D)
                if on_act:
                    nc.scalar.copy(dst, src_)
                else:
                    nc.vector.tensor_copy(dst, src_)

            units = [(unit, i, n) for i in range(KC) for n in range(4)]
            return vt, units

        def proj_v(pool, kv8, wv_t, on_act=False):
            vt, units = proj_v_units(pool, kv8, wv_t, on_act)
            for u, i, n in units:
                u(i, n)
            return vt

        def proj_kf_units(pool, kv8, wk_t, kb, on_act):
            """K projection units, d-folded output kf[32b+p, a, g, s]."""
            kf = pool.tile([P, C2, 2, S], F8, tag="kf", bufs=2)

            def unit(a, g, n2):
                pk = ps_mm()
                for c2 in range(C2):
                    nc.tensor.matmul(
                        pk[:], wk_t[:, c2, a, g, :, :],
                        kv8[:, c2, :, n2 * T:(n2 + 1) * T],
                        start=(c2 == 0), stop=(c2 == C2 - 1), perf_mode=DR)
                col = kb + 2 * a + g
                dst = kf[:, a, g, n2 * T:(n2 + 1) * T]
                if on_act:
                    nc.scalar.activation(dst, pk[:], AF.Identity,
                                         bias=vec[:, col:col + 1])
                else:
                    nc.vector.tensor_scalar(dst, pk[:], vec[:, col:col + 1],
                                            None, op0=OP.add)

            units = [(unit, a, g, n2) for a in range(4) for g in range(2)
                     for n2 in range(2)]
            return kf, units

        def proj_kf(pool, kv8, wk_t, kb, on_act):
            kf, units = proj_kf_units(pool, kv8, wk_t, kb, on_act)
            for u, a, g, n2 in units:
                u(a, g, n2)
            return kf

        def proj_qf(pool, q8, wq_t, qb, on_act=True):
            qf = pool.tile([P, C2, 2, T], F8, tag="qf")
            for a in range(4):
                for g in range(2):
                    pq = ps_mm()
                    for c2 in range(C2):
                        nc.tensor.matmul(
                            pq[:], wq_t[:, c2, a, g, :, :], q8[:, c2, :, :],
                            start=(c2 == 0), stop=(c2 == C2 - 1), perf_mode=DR)
                    col = qb + 2 * a + g
                    if on_act:
                        nc.scalar.activation(qf[:, a, g, :], pq[:],
                                             AF.Identity,
                                             bias=vec[:, col:col + 1])
                    else:
                        nc.vector.tensor_scalar(qf[:, a, g, :], pq[:],
                                                vec[:, col:col + 1], None,
                                                op0=OP.add)
            return qf

        def scores_av(pool, kf, qf, vt, ab, hooks=None):
            """Per-head softmax(scores)@V -> at8 [P, k2, fold, T] fp8."""
            at8 = pool.tile([P, C2, 2, T], F8, tag="at8")
            hidx = 0
            for m in range(KC):
                for h2 in (1, 0):
                    if hooks and hidx in hooks:
                        hooks[hidx]()
                    hidx += 1
                    h = 2 * m + h2
                    a, b_ = h // 4, h % 4
                    psAv = ps_av()
                    for i2 in range(C2):
                        sc = ps_sc()
                        for j in (0, 1):
                            i = 2 * i2 + j
                            nc.tensor.matmul(
                                sc[:, j, :],
                                kf[32 * b_:32 * b_ + 32, a, :,
                                   i * P:(i + 1) * P],
                                qf[32 * b_:32 * b_ + 32, a, :, :],
                                start=True, stop=True, perf_mode=DR,
                                tile_position=(32 * b_, 0))
                        et = pool.tile([P, 2, T], F8, tag="et", bufs=4)
                        if masked:
                            for j in (0, 1):
                                col = ab + 2 * i2 + j
                                nc.scalar.activation(
                                    et[:, j, :], sc[:, j, :], AF.Exp,
                                    bias=vec[:, col:col + 1],
                                    scale=1.0 / (SCALE * SCALE))
                        else:
                            nc.scalar.activation(et[:], sc[:], AF.Exp,
                                                 scale=1.0 / (SCALE * SCALE))
                        nc.tensor.matmul(psAv[0:D + 1, :],
                                         vt[:, i2, :, h, 0:D + 1], et[:],
                                         start=(i2 == 0), stop=(i2 == C2 - 1),
                                         perf_mode=DR)
                    rden = pool.tile([1, T], F32R, tag="rden", bufs=2)
                    nc.vector.reciprocal(rden[:], psAv[D:D + 1, :])
                    psB = ps_av()
                    nc.tensor.matmul(psB[0:D, :], ones2[0:1, 0:D], rden[:],
                                     start=True, stop=True)
                    rb = pool.tile([D, T], F32, tag="rb", bufs=2)
                    nc.vector.tensor_copy(rb[:], psB[0:D, :])
                    if h2 == 0:
                        nc.vector.tensor_tensor(
                            at8[0:D, m // 2, m % 2, :], psAv[0:D, :],
                            rb[:], op=OP.mult)
                    else:
                        atmp = pool.tile([D, T], F8, tag="atmp", bufs=2)
                        nc.vector.tensor_tensor(atmp[:], psAv[0:D, :],
                                                rb[:], op=OP.mult)
                        nc.gpsimd.dma_start(at8[D:P, m // 2, m % 2, :],
                                            atmp[:])
            return at8

        def proj_o(pool, at8, wo_t, ob, resid, out_f8=None, out_f32=None,
                   ln_hook=None):
            """Out-proj + bias' + residual; optional per-chunk LN-sum hook."""
            for m in range(KC):
                po = ps_mm()
                for k2 in range(C2):
                    nc.tensor.matmul(po[:], wo_t[:, k2, :, m, :],
                                     at8[:, k2, :, :],
                                     start=(k2 == 0), stop=(k2 == C2 - 1),
                                     perf_mode=DR)
                dst = (out_f8[:, m // 2, m % 2, :] if out_f8 is not None
                       else out_f32[:, m, :])
                nc.vector.scalar_tensor_tensor(
                    dst, po[:], vec[:, ob + m:ob + m + 1], resid[:, m, :],
                    op0=OP.add, op1=OP.add)
                if ln_hook is not None:
                    ln_hook(m)

        # ======================== layernorm helpers ========================
        def ln_math(pool, psS, psQ):
            """[1,T] sums -> sc tile with rstd bcast [:,0,:], mean bcast
            [:,1,:]."""
            mean = pool.tile([1, T], F32, tag="lnv", bufs=3)
            nc.scalar.mul(mean[:], psS, 1.0 / H)
            ex2 = pool.tile([1, T], F32, tag="lnv", bufs=3)
            nc.scalar.mul(ex2[:], psQ, 1.0 / H)
            var = pool.tile([1, T], F32, tag="lnv", bufs=3)
            nc.vector.tensor_tensor(var[:], mean[:], mean[:], op=OP.mult)
            nc.vector.tensor_tensor(var[:], ex2[:], var[:], op=OP.subtract)
            lv = pool.tile([1, T], F32, tag="lnv", bufs=3)
            nc.scalar.activation(lv[:], var[:], AF.Ln,
                                 bias=vec[0:1, C_EPS:C_EPS + 1])
            rstd = pool.tile([1, T], F32R, tag="lnr", bufs=2)
            nc.scalar.activation(rstd[:], lv[:], AF.Exp, scale=-0.5)
            meanr = pool.tile([1, T], F32R, tag="lnr", bufs=2)
            nc.vector.tensor_copy(meanr[:], mean[:])
            scB = ps_sc()
            nc.tensor.matmul(scB[:, 0, :], ones2[0:1, :], rstd[:],
                             start=True, stop=True)
            nc.tensor.matmul(scB[:, 1, :], ones2[0:1, :], meanr[:],
                             start=True, stop=True)
            sbB = pool.tile([P, 2, T], F32, tag="sbB", bufs=1)
            nc.vector.tensor_copy(sbB[:], scB[:])
            return sbB

        def ln_stats_f8(pool, src8):
            """LN sums from an fp8-folded [P, C2, 2, T] tensor."""
            scS = ps_sc()
            for j in range(C2):
                nc.tensor.matmul(scS[0:1, 0, :], ones8[:, :, 0:1],
                                 src8[:, j, :, :], start=(j == 0),
                                 stop=(j == C2 - 1), perf_mode=DR,
                                 skip_group_check=True)
            for j in range(C2):
                sq8 = pool.tile([P, 2, T], F8, tag="sq8", bufs=2)
                nc.gpsimd.tensor_tensor(sq8[:], src8[:, j, :, :],
                                        src8[:, j, :, :], op=OP.mult)
                nc.tensor.matmul(scS[0:1, 1, :], ones8[:, :, 0:1], sq8[:],
                                 start=(j == 0), stop=(j == C2 - 1),
                                 perf_mode=DR, skip_group_check=True)
            return scS

        def ln_dst(pool, src_ap, sbB, m, dst_ap):
            """dst = g*(src - mean)*rstd + b for one [P, T] chunk."""
            e1 = nc.gpsimd if m % 2 == 0 else nc.vector
            e2 = nc.vector if m % 2 == 0 else nc.gpsimd
            t1 = pool.tile([P, T], F32, tag="t1", bufs=3)
            e1.tensor_tensor(t1[:], src_ap, sbB[:, 1, :], op=OP.subtract)
            if gbtriv:
                e2.tensor_tensor(dst_ap, t1[:], sbB[:, 0, :], op=OP.mult)
            else:
                e2.tensor_tensor(t1[:], t1[:], sbB[:, 0, :], op=OP.mult)
                e1.tensor_scalar(dst_ap, t1[:], vec[:, C_G + m:C_G + m + 1],
                                 vec[:, C_B + m:C_B + m + 1], op0=OP.mult,
                                 op1=OP.add)

        # =========================== main program ==========================
        with tc.tile_pool(name="attn", bufs=1) as pool:
            def load_w(tag, shape, dram):
                t = pool.tile(shape, F8, tag=tag, bufs=1, name=tag)
                nc.sync.dma_start(t[:], dram[:])
                return t

            # V-proj weights first (first consumer of xk8), then stream xk
            wv_s = load_w("wv", [P, C2, 2, H], w8["sv"])
            xk8 = pool.tile([P, C2, 2, S], F8, tag="xk8")
            xkr = xk_d.rearrange("(c p) t -> p c t", p=P)
            for blk in range(8):
                stg = pool.tile([P, KC, P], F32, tag="stg", bufs=2)
                nc.sync.dma_start(stg[:], xkr[:, :, blk * P:(blk + 1) * P])
                nc.gpsimd.tensor_copy(
                    xk8.rearrange("p c f (b t) -> p c f b t", t=P)
                    [:, :, :, blk, :],
                    stg.rearrange("p (c f) t -> p c f t", f=2)[:])
            # K weights + query slice after the hidden stream
            wk_s = pool.tile([P, C2, 4, 2, 2, P], F8, tag="wk", bufs=1,
                             name="wk")
            nc.sync.dma_start(wk_s[:, :, 0:2, :, :, :],
                              w8["skf"][:, :, 0:2, :, :, :])
            xq = pool.tile([P, KC, T], F32, tag="xq")
            nc.sync.dma_start(xq[:], xq_d.rearrange("(c p) t -> p c t", p=P))
            xq8 = pool.tile([P, C2, 2, T], F8, tag="xq8")
            for c in range(KC):
                nc.gpsimd.tensor_copy(xq8[:, c // 2, c % 2, :], xq[:, c, :])
            wq_s = pool.tile([P, C2, 4, 2, 2, P], F8, tag="wq", bufs=1,
                             name="wq")
            nc.sync.dma_start(wq_s[:, :, 0:2, :, :, :],
                              w8["sqf"][:, :, 0:2, :, :, :])
            nc.sync.dma_start(wk_s[:, :, 2:4, :, :, :],
                              w8["skf"][:, :, 2:4, :, :, :])
            nc.sync.dma_start(wq_s[:, :, 2:4, :, :, :],
                              w8["sqf"][:, :, 2:4, :, :, :])
            wo_s = load_w("wo", [P, C2, 2, KC, P], w8["so"])

            # ---- cross hidden staging + fold (early; overlaps self attn) ----
            xc8 = pool.tile([P, C2, 2, S], F8, tag="xc8")
            xcr = xc_d.rearrange("(c p) t -> p c t", p=P)
            for j in range(KC):
                stg = pool.tile([P, S], F32, tag="stg", bufs=2)
                nc.sync.dma_start(stg[:], xcr[:, j, :])
                nc.gpsimd.tensor_copy(xc8[:, j // 2, j % 2, :], stg[:])
            # cross weights stream while self attention computes
            wv_c = load_w("wv", [P, C2, 2, H], w8["cv"])
            wk_c = load_w("wk", [P, C2, 4, 2, 2, P], w8["ckf"])
            wq_c = load_w("wq", [P, C2, 4, 2, 2, P], w8["cqf"])
            wo_c = load_w("wo", [P, C2, 2, KC, P], w8["co"])

            vt = proj_v(pool, xk8, wv_s, on_act=False)
            kf = proj_kf(pool, xk8, wk_s, C_SBK, on_act=False)
            qf = proj_qf(pool, xq8, wq_s, C_SBQ, on_act=False)
            vt_c, v_units = proj_v_units(pool, xc8, wv_c)
            kf_c, k_units = proj_kf_units(pool, xc8, wk_c, C_CBK,
                                          on_act=False)
            work = ([("v",) + u[1:] for u in v_units]
                    + [("k",) + u[1:] for u in k_units])
            v_unit, k_unit = v_units[0][0], k_units[0][0]
            hooks = {}
            for hi in range(2, 14):
                lo = (hi - 2) * 4
                chunk = work[lo:lo + 4]

                def _mk(chunk):
                    def _h():
                        for w in chunk:
                            if w[0] == "v":
                                v_unit(w[1], w[2])
                            else:
                                k_unit(w[1], w[2], w[3])
                    return _h

                hooks[hi] = _mk(chunk)
            at8 = scores_av(pool, kf, qf, vt, C_SAB, hooks=hooks)
            if dbg:
                nc.sync.dma_start(dbg_d["d_kf"][:], kf[:])
                nc.sync.dma_start(dbg_d["d_qf"][:], qf[:])
                nc.sync.dma_start(dbg_d["d_vt"][:], vt[:])
                nc.sync.dma_start(dbg_d["d_at"][:], at8[:])
            sa8 = pool.tile([P, C2, 2, T], F8, tag="sa8")
            acc1 = {}

            def ln1_hook(m):
                if m % 2 == 0:
                    return
                j = m // 2
                if "scS" not in acc1:
                    acc1["scS"] = ps_sc()
                scS = acc1["scS"]
                nc.tensor.matmul(scS[0:1, 0, :], ones8[:, :, 0:1],
                                 sa8[:, j, :, :], start=(j == 0),
                                 stop=(j == C2 - 1), perf_mode=DR,
                                 skip_group_check=True)
                sq8 = pool.tile([P, 2, T], F8, tag="sq8", bufs=2)
                nc.gpsimd.tensor_tensor(sq8[:], sa8[:, j, :, :],
                                        sa8[:, j, :, :], op=OP.mult)
                nc.tensor.matmul(scS[0:1, 1, :], ones8[:, :, 0:1], sq8[:],
                                 start=(j == 0), stop=(j == C2 - 1),
                                 perf_mode=DR, skip_group_check=True)

            proj_o(pool, at8, wo_s, C_SBO, xq, out_f8=sa8, ln_hook=ln1_hook)
            if dbg:
                nc.sync.dma_start(dbg_d["d_sa"][:], sa8[:])

            # ---- LN1 stats + fused cross-Q ----
            # q_c = rstd * (sa@Wq' - mean*colsum(Wq')) + bias'; the Wq'
            # matmuls consume sa8 directly, overlapping the LN1 math.
            scS1 = acc1["scS"]
            qf_c = pool.tile([P, C2, 2, T], F8, tag="qf")
            scB1 = None
            for a in range(4):
                for g in range(2):
                    i_ = 2 * a + g
                    pq = ps_mm()
                    for c2 in range(C2):
                        nc.tensor.matmul(
                            pq[:], wq_c[:, c2, a, g, :, :], sa8[:, c2, :, :],
                            start=(c2 == 0), stop=(c2 == C2 - 1), perf_mode=DR)
                    if scB1 is None:
                        scB1 = ln_math(pool, scS1[0:1, 0, :],
                                       scS1[0:1, 1, :])
                    t1 = pool.tile([P, T], F32, tag="t1", bufs=3)
                    nc.vector.scalar_tensor_tensor(
                        t1[:], scB1[:, 1, :],
                        vec[:, C_CQS + i_:C_CQS + i_ + 1], pq[:],
                        op0=OP.mult, op1=OP.add)
                    nc.vector.tensor_tensor(t1[:], t1[:], scB1[:, 0, :],
                                            op=OP.mult)
                    nc.scalar.activation(qf_c[:, a, g, :], t1[:], AF.Identity,
                                         bias=vec[:, C_CBQ + i_:
                                                  C_CBQ + i_ + 1])
            at8_c = scores_av(pool, kf_c, qf_c, vt_c, C_CAB)
            ca = pool.tile([P, KC, T], F32R, tag="ca")
            acc2 = {}

            def ln2_hook(m):
                # ride LN2 sums on ca chunks as they complete
                if "scS" not in acc2:
                    acc2["scS"] = ps_sc()
                scS = acc2["scS"]
                nc.tensor.matmul(scS[0:1, 0, :], ones2[:, 0:1], ca[:, m, :],
                                 start=(m == 0), stop=(m == KC - 1),
                                 skip_group_check=True)
                sq = pool.tile([P, T], F32R, tag="sq", bufs=2)
                nc.vector.tensor_tensor(sq[:], ca.bitcast(F32)[:, m, :],
                                        ca.bitcast(F32)[:, m, :], op=OP.mult)
                nc.tensor.matmul(scS[0:1, 1, :], ones2[:, 0:1], sq[:],
                                 start=(m == 0), stop=(m == KC - 1),
                                 skip_group_check=True)

            proj_o(pool, at8_c, wo_c, C_CBO, xq, out_f32=ca,
                   ln_hook=ln2_hook)

            # ---- LN2 (on ca) -> h (f32) + hf8 ----
            scS2 = acc2["scS"]
            scB2 = ln_math(pool, scS2[0:1, 0, :], scS2[0:1, 1, :])
            for m in range(KC):
                ln_dst(pool, ca.bitcast(F32)[:, m, :], scB2, m, h_t[:, m, :])
                nc.scalar.copy(hf8[:, m // 2, m % 2, :], h_t[:, m, :])
                nc.gpsimd.tensor_tensor(hl8[:, m // 2, m % 2, :],
                                        h_t[:, m, :],
                                        hf8[:, m // 2, m % 2, :],
                                        op=OP.subtract)
                nc.scalar.mul(hs8[:, m // 2, m % 2, :], h_t[:, m, :],
                              1.0 / 64.0)
            if dbg:
                nc.sync.dma_start(
                    dbg_d["d_h"].rearrange("(c p) t -> p c t", p=P), h_t[:])

        # ================= FFN (fp8 DoubleRow) + final LN ==================
        with tc.tile_pool(name="ffn", bufs=1) as pool:
            ut8 = pool.tile([P, 16, 2, T], F8, tag="ut8")
            us8 = pool.tile([P, 16, 2, T], F8, tag="us8")
            for m0 in range(0, 32, 8):
                w1t = pool.tile([P, 2, C2, 2, 8, P], F8, tag="w1t", bufs=2)
                nc.sync.dma_start(w1t[:], w18_d[:, m0 // 8, :, :, :, :, :])
                for m in range(m0, m0 + 8):
                    pu = ps_mm()
                    passes = [(0, hf8), (0, hl8), (1, hs8)]
                    for pi, (lo, hsrc) in enumerate(passes):
                        for c2 in range(C2):
                            nc.tensor.matmul(
                                pu[:], w1t[:, lo, c2, :, m - m0, :],
                                hsrc[:, c2, :, :],
                                start=(pi == 0 and c2 == 0),
                                stop=(pi == 2 and c2 == C2 - 1),
                                perf_mode=DR)
                    nc.vector.tensor_scalar(ut8[:, m // 2, m % 2, :], pu[:],
                                      vec[:, C_B1 + m:C_B1 + m + 1], 0.0,
                                      op0=OP.add, op1=OP.max)
                    nc.gpsimd.tensor_scalar(us8[:, m // 2, m % 2, :],
                                            ut8[:, m // 2, m % 2, :],
                                            1.0 / 64.0, None, op0=OP.mult)
            if dbg:
                nc.sync.dma_start(dbg_d["d_u"][:], ut8[:])

            ff = pool.tile([P, KC, T], F32R, tag="ff")
            acc3 = {}
            for m in range(KC):
                w2t = pool.tile([P, 2, 16, 2, P], F8, tag="w2t", bufs=3)
                nc.sync.dma_start(w2t[:], w28_d[:, m, :, :, :, :])
                pf = ps_mm()
                for lo, usrc in ((0, ut8), (1, us8)):
                    for k2 in range(16):
                        nc.tensor.matmul(pf[:], w2t[:, lo, k2, :, :],
                                         usrc[:, k2, :, :],
                                         start=(lo == 0 and k2 == 0),
                                         stop=(lo == 1 and k2 == 15),
                                         perf_mode=DR)
                nc.vector.scalar_tensor_tensor(
                    ff[:, m, :], pf[:], vec[:, C_B2 + m:C_B2 + m + 1],
                    h_t[:, m, :], op0=OP.add, op1=OP.add)
                if "scS" not in acc3:
                    acc3["scS"] = ps_sc()
                scS = acc3["scS"]
                nc.tensor.matmul(scS[0:1, 0, :], ones2[:, 0:1], ff[:, m, :],
                                 start=(m == 0), stop=(m == KC - 1),
                                 skip_group_check=True)
                sq = pool.tile([P, T], F32R, tag="sq", bufs=2)
                nc.gpsimd.tensor_tensor(sq[:], ff.bitcast(F32)[:, m, :],
                                        ff.bitcast(F32)[:, m, :], op=OP.mult)
                nc.tensor.matmul(scS[0:1, 1, :], ones2[:, 0:1], sq[:],
                                 start=(m == 0), stop=(m == KC - 1),
                                 skip_group_check=True)

            scS3 = acc3["scS"]
            scB3 = ln_math(pool, scS3[0:1, 0, :], scS3[0:1, 1, :])
            for m in range(KC):
                ob = pool.tile([P, T], F32, tag="ob", bufs=3)
                ln_dst(pool, ff.bitcast(F32)[:, m, :], scB3, m, ob[:])
                nc.sync.dma_start(out_d[m * P:(m + 1) * P, :], ob[:])

    _legalize_waits(nc)
    return nc


# revision 47
# speedup vs baseline: 1.5315x; 1.0147x over previous
"""Trainium2 Bass kernel for nn_Block_30107720745811 (dense transformer block).

B=4, S=1024, H=1024, NH=16. 8 NeuronCores, zero-communication sharding:
core c computes batch b=c//2, query rows (c%2)*512:(c%2)*512+512.

All big GEMMs run as fp8e4m3 DoubleRow matmuls (0.5 cycles/row with a packed
256-deep contraction => 4x the fp32r PE rate). Activations live feature-major
[feature, token]; fp8 operands use the DoubleRow fold layout [128, 2, N]
(features 256c+128f+p at [p, f, n]). Q/K projections emit a d-folded layout
(head h=4a+b dim 32g+p at kf[32b+p, a, g, t]) via host-permuted weight
columns, so the per-head scores GEMM is also DoubleRow (K_p=32). The softmax
exp converts straight to fp8 folded tiles consumed by the att@V DoubleRow.
Residual-path tensors (xq, ca, h, ff) stay f32 for precision; the softmax
denominator rides att@V as a ones column of V.
"""
import numpy as np
import ml_dtypes
import concourse.bass as bass
import concourse.tile as tile
from concourse import mybir
from concourse import bass_utils
from concourse.alu_op_type import AluOpType as OP

AF = mybir.ActivationFunctionType
F32 = mybir.dt.float32
F32R = mybir.dt.float32r
F8 = mybir.dt.float8e4
DR = mybir.MatmulPerfMode.DoubleRow
FP8 = ml_dtypes.float8_e4m3

B, S, H, NH = 4, 1024, 1024, 16
D = H // NH          # 64
P = 128
T = 512              # query tokens per core
KC = H // P          # 8 feature chunks
C2 = 4               # 256-wide fold chunks
NS = 256             # V-proj output slice (4 heads)
VP = 80              # padded per-head vt row (D+1 used, 16B-aligned pitch)
INF = 1e10
EPS = 1e-5
SCALE = 8.0

# vec tensor column map
C_SBQ, C_SBK, C_SBO = 0, 8, 16
C_CBQ, C_CBK, C_CBO = 24, 32, 40
C_SAB, C_CAB = 48, 56
C_G, C_B = 64, 72
C_B1, C_B2, C_EPS = 80, 112, 120
C_CQS = 121
NVEC = 129

MAX_WAITS = 1


def _legalize_waits(nc, max_waits=MAX_WAITS):
    """Split >max_waits semaphore waits into preceding same-engine NOPs
    (this walrus build allows only one sync wait per instruction)."""
    n_split = 0
    for f in nc.m.functions:
        for blk in f.blocks:
            out = []
            for ins in blk.instructions:
                si = getattr(ins, "sync_info", None)
                if si is not None and si.on_wait and len(si.on_wait) > max_waits:
                    waits = list(si.on_wait)
                    extra, keep = waits[:-max_waits], waits[-max_waits:]
                    for j in range(0, len(extra), max_waits):
                        out.append(mybir.InstNoOp(
                            name=f"{ins.name}-lw{j}",
                            engine=ins.engine,
                            sync_info=mybir.SyncInfo(
                                on_wait=extra[j:j + max_waits], on_update=[]),
                            bass_nofuse=True,
                        ))
                    ins.sync_info = mybir.SyncInfo(
                        on_wait=keep, on_update=list(si.on_update))
                    n_split += 1
                out.append(ins)
            blk.instructions = out
    return n_split


def _build(dbg=False, masked=False, gbtriv=True):
    nc = bass.Bass("TRN2", target_bir_lowering=False, debug=False,
                   dynamic_dma_scratch_size=8192)

    def din(name, shape, dt=F32):
        return nc.dram_tensor(name, shape, dt, kind="ExternalInput").ap()

    xk_d = din("xkT", [H, S])            # self hidden, transposed
    xc_d = din("xcT", [H, S])            # cross hidden, transposed
    xq_d = din("xqT", [H, T])            # query cols of self hidden
    w8 = {}
    for p_ in ("s", "c"):
        w8[p_ + "qf"] = din(p_ + "WqF", [P, C2, 4, 2, 2, P], F8)
        w8[p_ + "kf"] = din(p_ + "WkF", [P, C2, 4, 2, 2, P], F8)
        w8[p_ + "v"] = din(p_ + "Wv8", [P, C2, 2, H], F8)
        w8[p_ + "o"] = din(p_ + "Wo8", [P, C2, 2, KC, P], F8)
    w18_d = din("W18", [P, 4, 2, C2, 2, 8, P], F8)  # m-blk, hi/lo outermost
    w28_d = din("W28", [P, KC, 2, 16, 2, P], F8)    # out-chunk m, hi/lo
    vec_d = din("vec", [P, NVEC], F32)
    ones2_d = din("ones2", [P, P], F32R)
    out_d = nc.dram_tensor("out", [H, T], F32, kind="ExternalOutput").ap()
    dbg_d = {}
    if dbg:
        for n, shape, dt in [("d_kf", [P, C2, 2, S], F8),
                             ("d_qf", [P, C2, 2, T], F8),
                             ("d_vt", [P, C2, 2, NH, VP], F8),
                             ("d_at", [P, C2, 2, T], F8),
                             ("d_sa", [P, C2, 2, T], F8),
                             ("d_snn", [P, C2, 2, T], F8),
                             ("d_h", [H, T], F32),
                             ("d_u", [P, 16, 2, T], F8)]:
            dbg_d[n] = nc.dram_tensor(n, shape, dt, kind="ExternalOutput").ap()

    with (
        tile.TileContext(nc) as tc,
        nc.allow_low_precision(reason="fp8 matmuls, f32 residual path"),
        tc.tile_pool(name="glob", bufs=1) as glob,
        tc.tile_pool(name="ps", bufs=1, space="PSUM") as ps,
    ):
        # ---- constants / persistent tiles ----
        vec = glob.tile([P, NVEC], F32, tag="vec")
        nc.sync.dma_start(vec[:], vec_d[:])
        ones2 = glob.tile([P, P], F32R, tag="ones2")
        nc.sync.dma_start(ones2[:], ones2_d[:])
        ones8 = glob.tile([P, 2, 16], F8, tag="ones8")
        nc.vector.memset(ones8[:], 1.0)
        h_t = glob.tile([P, KC, T], F32, tag="h")       # LN2 out (residual)
        hf8 = glob.tile([P, C2, 2, T], F8, tag="hf8")   # LN2 out fp8 (FFN1)
        hl8 = glob.tile([P, C2, 2, T], F8, tag="hl8")   # fp8 residual of h
        hs8 = glob.tile([P, C2, 2, T], F8, tag="hs8")   # h/64 for W1lo pass

        def ps_mm():
            return ps.tile([P, T], F32, tag="mm", bufs=2, name="psmm")

        def ps_sc():
            return ps.tile([P, 2, T], F32, tag="sc", bufs=2, name="pssc")

        def ps_av():
            return ps.tile([P, T], F32, tag="av", bufs=2, name="psav")

        # ============================ attention ============================
        def proj_v_units(pool, kv8, wv_t, on_act=False):
            """V projection units -> vt [P, i2, fold, head, 0:D] fp8."""
            vt = pool.tile([P, C2, 2, NH, VP], F8, tag="vt", bufs=2)
            nc.vector.memset(vt[:, :, :, :, D:D + 1], 1.0)

            def unit(i, n):
                pv = ps_mm()
                for c2 in range(C2):
                    nc.tensor.matmul(
                        pv[:, 0:NS], kv8[:, c2, :, i * P:(i + 1) * P],
                        wv_t[:, c2, :, n * NS:(n + 1) * NS],
                        start=(c2 == 0), stop=(c2 == C2 - 1), perf_mode=DR)
                dst = vt[:, i // 2, i % 2, 4 * n:4 * (n + 1), 0:D]
                src_ = pv[:, 0:NS].rearrange("p (h d) -> p h d", d=D)
                if on_act:
                    nc.scalar.copy(dst, src_)
                else:
                    nc.vector.tensor_copy(dst, src_)

            units = [(unit, i, n) for i in range(KC) for n in range(4)]
            return vt, units

        def proj_v(pool, kv8, wv_t, on_act=False):
            vt, units = proj_v_units(pool, kv8, wv_t, on_act)
            for u, i, n in units:
                u(i, n)
            return vt

        def proj_kf_units(pool, kv8, wk_t, kb, on_act):
            """K projection units, d-folded output kf[32b+p, a, g, s]."""
            kf = pool.tile([P, C2, 2, S], F8, tag="kf", bufs=2)

            def unit(a, g, n2):
                pk = ps_mm()
                for c2 in range(C2):
                    nc.tensor.matmul(
                        pk[:], wk_t[:, c2, a, g, :, :],
                        kv8[:, c2, :, n2 * T:(n2 + 1) * T],
                        start=(c2 == 0), stop=(c2 == C2 - 1), perf_mode=DR)
                col = kb + 2 * a + g
                dst = kf[:, a, g, n2 * T:(n2 + 1) * T]
                if on_act:
                    nc.scalar.activation(dst, pk[:], AF.Identity,
                                         bias=vec[:, col:col + 1])
                else:
                    nc.vector.tensor_scalar(dst, pk[:], vec[:, col:col + 1],
                                            None, op0=OP.add)

            units = [(unit, a, g, n2) for a in range(4) for g in range(2)
                     for n2 in range(2)]
            return kf, units

        def proj_kf(pool, kv8, wk_t, kb, on_act):
            kf, units = proj_kf_units(pool, kv8, wk_t, kb, on_act)
            for u, a, g, n2 in units:
                u(a, g, n2)
            return kf

        def proj_qf(pool, q8, wq_t, qb, on_act=True):
            qf = pool.tile([P, C2, 2, T], F8, tag="qf")
            for a in range(4):
                for g in range(2):
                    pq = ps_mm()
                    for c2 in range(C2):
                        nc.tensor.matmul(
                            pq[:], wq_t[:, c2, a, g, :, :], q8[:, c2, :, :],
                            start=(c2 == 0), stop=(c2 == C2 - 1), perf_mode=DR)
                    col = qb + 2 * a + g
                    if on_act:
                        nc.scalar.activation(qf[:, a, g, :], pq[:],
                                             AF.Identity,
                                             bias=vec[:, col:col + 1])
                    else:
                        nc.vector.tensor_scalar(qf[:, a, g, :], pq[:],
                                                vec[:, col:col + 1], None,
                                                op0=OP.add)
            return qf

        def scores_av(pool, kf, qf, vt, ab, hooks=None):
            """Per-head softmax(scores)@V -> at8 [P, k2, fold, T] fp8."""
            at8 = pool.tile([P, C2, 2, T], F8, tag="at8")
            hidx = 0
            for m in range(KC):
                for h2 in (1, 0):
                    if hooks and hidx in hooks:
                        hooks[hidx]()
                    hidx += 1
                    h = 2 * m + h2
                    a, b_ = h // 4, h % 4
                    psAv = ps_av()
                    for i2 in range(C2):
                        sc = ps_sc()
                        for j in (0, 1):
                            i = 2 * i2 + j
                            nc.tensor.matmul(
                                sc[:, j, :],
                                kf[32 * b_:32 * b_ + 32, a, :,
                                   i * P:(i + 1) * P],
                                qf[32 * b_:32 * b_ + 32, a, :, :],
                                start=True, stop=True, perf_mode=DR,
                                tile_position=(32 * b_, 0))
                        et = pool.tile([P, 2, T], F8, tag="et", bufs=4)
                        if masked:
                            for j in (0, 1):
                                col = ab + 2 * i2 + j
                                nc.scalar.activation(
                                    et[:, j, :], sc[:, j, :], AF.Exp,
                                    bias=vec[:, col:col + 1],
                                    scale=1.0 / (SCALE * SCALE))
                        else:
                            nc.scalar.activation(et[:], sc[:], AF.Exp,
                                                 scale=1.0 / (SCALE * SCALE))
                        nc.tensor.matmul(psAv[0:D + 1, :],
                                         vt[:, i2, :, h, 0:D + 1], et[:],
                                         start=(i2 == 0), stop=(i2 == C2 - 1),
                                         perf_mode=DR)
                    rden = pool.tile([1, T], F32R, tag="rden", bufs=2)
                    nc.vector.reciprocal(rden[:], psAv[D:D + 1, :])
                    psB = ps_av()
                    nc.tensor.matmul(psB[0:D, :], ones2[0:1, 0:D], rden[:],
                                     start=True, stop=True)
                    rb = pool.tile([D, T], F32, tag="rb", bufs=2)
                    nc.vector.tensor_copy(rb[:], psB[0:D, :])
                    if h2 == 0:
                        nc.vector.tensor_tensor(
                            at8[0:D, m // 2, m % 2, :], psAv[0:D, :],
                            rb[:], op=OP.mult)
                    else:
                        atmp = pool.tile([D, T], F8, tag="atmp", bufs=1)
                        nc.vector.tensor_tensor(atmp[:], psAv[0:D, :],
                                                rb[:], op=OP.mult)
                        nc.gpsimd.dma_start(at8[D:P, m // 2, m % 2, :],
                                            atmp[:])
            return at8

        def proj_o(pool, at8, wo_t, ob, resid, out_f8=None, out_f32=None,
                   ln_hook=None):
            """Out-proj + bias' + residual; optional per-chunk LN-sum hook."""
            for m in range(KC):
                po = ps_mm()
                for k2 in range(C2):
                    nc.tensor.matmul(po[:], wo_t[:, k2, :, m, :],
                                     at8[:, k2, :, :],
                                     start=(k2 == 0), stop=(k2 == C2 - 1),
                                     perf_mode=DR)
                dst = (out_f8[:, m // 2, m % 2, :] if out_f8 is not None
                       else out_f32[:, m, :])
                nc.vector.scalar_tensor_tensor(
                    dst, po[:], vec[:, ob + m:ob + m + 1], resid[:, m, :],
                    op0=OP.add, op1=OP.add)
                if ln_hook is not None:
                    ln_hook(m)

        # ======================== layernorm helpers ========================
        def ln_math(pool, psS, psQ):
            """[1,T] sums -> sc tile with rstd bcast [:,0,:], mean bcast
            [:,1,:]."""
            mean = pool.tile([1, T], F32, tag="lnv", bufs=3)
            nc.scalar.mul(mean[:], psS, 1.0 / H)
            ex2 = pool.tile([1, T], F32, tag="lnv", bufs=3)
            nc.scalar.mul(ex2[:], psQ, 1.0 / H)
            var = pool.tile([1, T], F32, tag="lnv", bufs=3)
            nc.vector.tensor_tensor(var[:], mean[:], mean[:], op=OP.mult)
            nc.vector.tensor_tensor(var[:], ex2[:], var[:], op=OP.subtract)
            lv = pool.tile([1, T], F32, tag="lnv", bufs=3)
            nc.scalar.activation(lv[:], var[:], AF.Ln,
                                 bias=vec[0:1, C_EPS:C_EPS + 1])
            rstd = pool.tile([1, T], F32R, tag="lnr", bufs=2)
            nc.scalar.activation(rstd[:], lv[:], AF.Exp, scale=-0.5)
            meanr = pool.tile([1, T], F32R, tag="lnr", bufs=2)
            nc.vector.tensor_copy(meanr[:], mean[:])
            scB = ps_sc()
            nc.tensor.matmul(scB[:, 0, :], ones2[0:1, :], rstd[:],
                             start=True, stop=True)
            nc.tensor.matmul(scB[:, 1, :], ones2[0:1, :], meanr[:],
                             start=True, stop=True)
            sbB = pool.tile([P, 2, T], F32, tag="sbB", bufs=1)
            nc.vector.tensor_copy(sbB[:], scB[:])
            return sbB

        def ln_stats_f8(pool, src8):
            """LN sums from an fp8-folded [P, C2, 2, T] tensor."""
            scS = ps_sc()
            for j in range(C2):
                nc.tensor.matmul(scS[0:1, 0, :], ones8[:, :, 0:1],
                                 src8[:, j, :, :], start=(j == 0),
                                 stop=(j == C2 - 1), perf_mode=DR,
                                 skip_group_check=True)
            for j in range(C2):
                sq8 = pool.tile([P, 2, T], F8, tag="sq8", bufs=2)
                nc.gpsimd.tensor_tensor(sq8[:], src8[:, j, :, :],
                                        src8[:, j, :, :], op=OP.mult)
                nc.tensor.matmul(scS[0:1, 1, :], ones8[:, :, 0:1], sq8[:],
                                 start=(j == 0), stop=(j == C2 - 1),
                                 perf_mode=DR, skip_group_check=True)
            return scS

        def ln_dst(pool, src_ap, sbB, m, dst_ap):
            """dst = g*(src - mean)*rstd + b for one [P, T] chunk."""
            e1 = nc.gpsimd if m % 2 == 0 else nc.vector
            e2 = nc.vector if m % 2 == 0 else nc.gpsimd
            t1 = pool.tile([P, T], F32, tag="t1", bufs=3)
            e1.tensor_tensor(t1[:], src_ap, sbB[:, 1, :], op=OP.subtract)
            if gbtriv:
                e2.tensor_tensor(dst_ap, t1[:], sbB[:, 0, :], op=OP.mult)
            else:
                e2.tensor_tensor(t1[:], t1[:], sbB[:, 0, :], op=OP.mult)
                e1.tensor_scalar(dst_ap, t1[:], vec[:, C_G + m:C_G + m + 1],
                                 vec[:, C_B + m:C_B + m + 1], op0=OP.mult,
                                 op1=OP.add)

        # =========================== main program ==========================
        with tc.tile_pool(name="attn", bufs=1) as pool:
            def load_w(tag, shape, dram):
                t = pool.tile(shape, F8, tag=tag, bufs=1, name=tag)
                nc.sync.dma_start(t[:], dram[:])
                return t

            # V-proj weights first (first consumer of xk8), then stream xk
            wv_s = load_w("wv", [P, C2, 2, H], w8["sv"])
            xk8 = pool.tile([P, C2, 2, S], F8, tag="xk8")
            xkr = xk_d.rearrange("(c p) t -> p c t", p=P)
            for blk in range(8):
                stg = pool.tile([P, KC, P], F32, tag="stg", bufs=2)
                nc.sync.dma_start(stg[:], xkr[:, :, blk * P:(blk + 1) * P])
                nc.gpsimd.tensor_copy(
                    xk8.rearrange("p c f (b t) -> p c f b t", t=P)
                    [:, :, :, blk, :],
                    stg.rearrange("p (c f) t -> p c f t", f=2)[:])
            # K weights + query slice after the hidden stream
            wk_s = pool.tile([P, C2, 4, 2, 2, P], F8, tag="wk", bufs=1,
                             name="wk")
            nc.sync.dma_start(wk_s[:, :, 0:2, :, :, :],
                              w8["skf"][:, :, 0:2, :, :, :])
            xq = pool.tile([P, KC, T], F32, tag="xq")
            nc.sync.dma_start(xq[:], xq_d.rearrange("(c p) t -> p c t", p=P))
            xq8 = pool.tile([P, C2, 2, T], F8, tag="xq8")
            for c in range(KC):
                nc.gpsimd.tensor_copy(xq8[:, c // 2, c % 2, :], xq[:, c, :])
            wq_s = pool.tile([P, C2, 4, 2, 2, P], F8, tag="wq", bufs=1,
                             name="wq")
            nc.sync.dma_start(wq_s[:, :, 0:2, :, :, :],
                              w8["sqf"][:, :, 0:2, :, :, :])
            nc.sync.dma_start(wk_s[:, :, 2:4, :, :, :],
                              w8["skf"][:, :, 2:4, :, :, :])
            nc.sync.dma_start(wq_s[:, :, 2:4, :, :, :],
                              w8["sqf"][:, :, 2:4, :, :, :])
            wo_s = load_w("wo", [P, C2, 2, KC, P], w8["so"])

            # ---- cross hidden staging + fold (early; overlaps self attn) ----
            xc8 = pool.tile([P, C2, 2, S], F8, tag="xc8")
            xcr = xc_d.rearrange("(c p) t -> p c t", p=P)
            for j in range(KC):
                stg = pool.tile([P, S], F32, tag="stg", bufs=2)
                nc.sync.dma_start(stg[:], xcr[:, j, :])
                nc.gpsimd.tensor_copy(xc8[:, j // 2, j % 2, :], stg[:])
            # cross weights stream while self attention computes
            wv_c = load_w("wv", [P, C2, 2, H], w8["cv"])
            wk_c = load_w("wk", [P, C2, 4, 2, 2, P], w8["ckf"])
            wq_c = load_w("wq", [P, C2, 4, 2, 2, P], w8["cqf"])
            wo_c = load_w("wo", [P, C2, 2, KC, P], w8["co"])
            w1t0 = glob.tile([P, 2, C2, 2, 4, P], F8, tag="w1t0")
            nc.sync.dma_start(w1t0[:], w18_d[:, 0, :, :, :, 0:4, :])

            vt = proj_v(pool, xk8, wv_s, on_act=False)
            kf = proj_kf(pool, xk8, wk_s, C_SBK, on_act=False)
            qf = proj_qf(pool, xq8, wq_s, C_SBQ, on_act=False)
            vt_c, v_units = proj_v_units(pool, xc8, wv_c)
            kf_c, k_units = proj_kf_units(pool, xc8, wk_c, C_CBK,
                                          on_act=False)
            work = ([("v",) + u[1:] for u in v_units]
                    + [("k",) + u[1:] for u in k_units])
            v_unit, k_unit = v_units[0][0], k_units[0][0]
            hooks = {}
            for hi in range(2, 14):
                lo = (hi - 2) * 4
                chunk = work[lo:lo + 4]

                def _mk(chunk):
                    def _h():
                        for w in chunk:
                            if w[0] == "v":
                                v_unit(w[1], w[2])
                            else:
                                k_unit(w[1], w[2], w[3])
                    return _h

                hooks[hi] = _mk(chunk)
            at8 = scores_av(pool, kf, qf, vt, C_SAB, hooks=hooks)
            if dbg:
                nc.sync.dma_start(dbg_d["d_kf"][:], kf[:])
                nc.sync.dma_start(dbg_d["d_qf"][:], qf[:])
                nc.sync.dma_start(dbg_d["d_vt"][:], vt[:])
                nc.sync.dma_start(dbg_d["d_at"][:], at8[:])
            sa8 = pool.tile([P, C2, 2, T], F8, tag="sa8")
            acc1 = {}

            def ln1_hook(m):
                if m % 2 == 0:
                    return
                j = m // 2
                if "scS" not in acc1:
                    acc1["scS"] = ps_sc()
                scS = acc1["scS"]
                nc.tensor.matmul(scS[0:1, 0, :], ones8[:, :, 0:1],
                                 sa8[:, j, :, :], start=(j == 0),
                                 stop=(j == C2 - 1), perf_mode=DR,
                                 skip_group_check=True)
                sq8 = pool.tile([P, 2, T], F8, tag="sq8", bufs=2)
                nc.gpsimd.tensor_tensor(sq8[:], sa8[:, j, :, :],
                                        sa8[:, j, :, :], op=OP.mult)
                nc.tensor.matmul(scS[0:1, 1, :], ones8[:, :, 0:1], sq8[:],
                                 start=(j == 0), stop=(j == C2 - 1),
                                 perf_mode=DR, skip_group_check=True)

            proj_o(pool, at8, wo_s, C_SBO, xq, out_f8=sa8, ln_hook=ln1_hook)
            if dbg:
                nc.sync.dma_start(dbg_d["d_sa"][:], sa8[:])

            # ---- LN1 stats + fused cross-Q ----
            # q_c = rstd * (sa@Wq' - mean*colsum(Wq')) + bias'; the Wq'
            # matmuls consume sa8 directly, overlapping the LN1 math.
            scS1 = acc1["scS"]
            qf_c = pool.tile([P, C2, 2, T], F8, tag="qf")
            scB1 = None
            for a in range(4):
                for g in range(2):
                    i_ = 2 * a + g
                    pq = ps_mm()
                    for c2 in range(C2):
                        nc.tensor.matmul(
                            pq[:], wq_c[:, c2, a, g, :, :], sa8[:, c2, :, :],
                            start=(c2 == 0), stop=(c2 == C2 - 1), perf_mode=DR)
                    if scB1 is None:
                        scB1 = ln_math(pool, scS1[0:1, 0, :],
                                       scS1[0:1, 1, :])
                    t1 = pool.tile([P, T], F32, tag="t1", bufs=3)
                    nc.vector.scalar_tensor_tensor(
                        t1[:], scB1[:, 1, :],
                        vec[:, C_CQS + i_:C_CQS + i_ + 1], pq[:],
                        op0=OP.mult, op1=OP.add)
                    nc.vector.tensor_tensor(t1[:], t1[:], scB1[:, 0, :],
                                            op=OP.mult)
                    nc.scalar.activation(qf_c[:, a, g, :], t1[:], AF.Identity,
                                         bias=vec[:, C_CBQ + i_:
                                                  C_CBQ + i_ + 1])
            at8_c = scores_av(pool, kf_c, qf_c, vt_c, C_CAB)
            ca = pool.tile([P, KC, T], F32R, tag="ca")
            acc2 = {}

            def ln2_hook(m):
                # ride LN2 sums on ca chunks as they complete
                if "scS" not in acc2:
                    acc2["scS"] = ps_sc()
                scS = acc2["scS"]
                nc.tensor.matmul(scS[0:1, 0, :], ones2[:, 0:1], ca[:, m, :],
                                 start=(m == 0), stop=(m == KC - 1),
                                 skip_group_check=True)
                sq = pool.tile([P, T], F32R, tag="sq", bufs=2)
                nc.vector.tensor_tensor(sq[:], ca.bitcast(F32)[:, m, :],
                                        ca.bitcast(F32)[:, m, :], op=OP.mult)
                nc.tensor.matmul(scS[0:1, 1, :], ones2[:, 0:1], sq[:],
                                 start=(m == 0), stop=(m == KC - 1),
                                 skip_group_check=True)

            proj_o(pool, at8_c, wo_c, C_CBO, xq, out_f32=ca,
                   ln_hook=ln2_hook)

            # ---- LN2 (on ca) -> h (f32) + hf8 ----
            scS2 = acc2["scS"]
            scB2 = ln_math(pool, scS2[0:1, 0, :], scS2[0:1, 1, :])
            for m in range(KC):
                ln_dst(pool, ca.bitcast(F32)[:, m, :], scB2, m, h_t[:, m, :])
                nc.scalar.copy(hf8[:, m // 2, m % 2, :], h_t[:, m, :])
                nc.gpsimd.tensor_tensor(hl8[:, m // 2, m % 2, :],
                                        h_t[:, m, :],
                                        hf8[:, m // 2, m % 2, :],
                                        op=OP.subtract)
                nc.scalar.mul(hs8[:, m // 2, m % 2, :], h_t[:, m, :],
                              1.0 / 64.0)
            if dbg:
                nc.sync.dma_start(
                    dbg_d["d_h"].rearrange("(c p) t -> p c t", p=P), h_t[:])

        # ================= FFN (fp8 DoubleRow) + final LN ==================
        with tc.tile_pool(name="ffn", bufs=1) as pool:
            ut8 = pool.tile([P, 16, 2, T], F8, tag="ut8")
            us8 = pool.tile([P, 16, 2, T], F8, tag="us8")
            for m0 in range(0, 32, 4):
                if m0 == 0:
                    w1t = w1t0
                else:
                    w1t = pool.tile([P, 2, C2, 2, 4, P], F8, tag="w1t",
                                    bufs=2)
                    q = m0 // 4
                    nc.sync.dma_start(
                        w1t[:], w18_d[:, q // 2, :, :, :,
                                      (q % 2) * 4:(q % 2) * 4 + 4, :])
                for m in range(m0, m0 + 4):
                    pu = ps_mm()
                    passes = [(0, hf8), (0, hl8), (1, hs8)]
                    for pi, (lo, hsrc) in enumerate(passes):
                        for c2 in range(C2):
                            nc.tensor.matmul(
                                pu[:], w1t[:, lo, c2, :, m - m0, :],
                                hsrc[:, c2, :, :],
                                start=(pi == 0 and c2 == 0),
                                stop=(pi == 2 and c2 == C2 - 1),
                                perf_mode=DR)
                    nc.vector.tensor_scalar(ut8[:, m // 2, m % 2, :], pu[:],
                                      vec[:, C_B1 + m:C_B1 + m + 1], 0.0,
                                      op0=OP.add, op1=OP.max)
                    nc.gpsimd.tensor_scalar(us8[:, m // 2, m % 2, :],
                                            ut8[:, m // 2, m % 2, :],
                                            1.0 / 64.0, None, op0=OP.mult)
            if dbg:
                nc.sync.dma_start(dbg_d["d_u"][:], ut8[:])

            ff = pool.tile([P, KC, T], F32R, tag="ff")
            acc3 = {}
            for m in range(KC):
                w2t = pool.tile([P, 2, 16, 2, P], F8, tag="w2t", bufs=3)
                nc.sync.dma_start(w2t[:], w28_d[:, m, :, :, :, :])
                pf = ps_mm()
                for lo, usrc in ((0, ut8), (1, us8)):
                    for k2 in range(16):
                        nc.tensor.matmul(pf[:], w2t[:, lo, k2, :, :],
                                         usrc[:, k2, :, :],
                                         start=(lo == 0 and k2 == 0),
                                         stop=(lo == 1 and k2 == 15),
                                         perf_mode=DR)
                nc.vector.scalar_tensor_tensor(
                    ff[:, m, :], pf[:], vec[:, C_B2 + m:C_B2 + m + 1],
                    h_t[:, m, :], op0=OP.add, op1=OP.add)
                if "scS" not in acc3:
                    acc3["scS"] = ps_sc()
                scS = acc3["scS"]
                nc.tensor.matmul(scS[0:1, 0, :], ones2[:, 0:1], ff[:, m, :],
                                 start=(m == 0), stop=(m == KC - 1),
                                 skip_group_check=True)
                sq = pool.tile([P, T], F32R, tag="sq", bufs=2)
                nc.gpsimd.tensor_tensor(sq[:], ff.bitcast(F32)[:, m, :],
                                        ff.bitcast(F32)[:, m, :], op=OP.mult)
                nc.tensor.matmul(scS[0:1, 1, :], ones2[:, 0:1], sq[:],
                                 start=(m == 0), stop=(m == KC - 1),
                                 skip_group_check=True)

            scS3 = acc3["scS"]
            scB3 = ln_math(pool, scS3[0:1, 0, :], scS3[0:1, 1, :])
            for m in range(KC):
                ob = pool.tile([P, T], F32, tag="ob", bufs=3)
                ln_dst(pool, ff.bitcast(F32)[:, m, :], scB3, m, ob[:])
                nc.sync.dma_start(out_d[m * P:(m + 1) * P, :], ob[:])

    _legalize_waits(nc)
    return nc
